# revision 1
# baseline (speedup 1.0000x reference)
"""Multi-head attention (B=2, S=2048, D=1024, H=16) on 8 Trainium2 cores.

Sharding: core c = (batch b, head-group hg) with b = c // 4, hg = c % 4.
Each core computes 4 heads of one batch element end-to-end:
  - Q^T/K^T projections in [dh, s] layout (scores computed transposed so the
    softmax denominator comes out of the PV matmul via a ones-column in V)
  - V projection in natural [s, dh] layout
  - exp on ScalarE with the 1/sqrt(dh) scale fused into the activation
  - partial output projection against the core's row-slice of Wo
Host sums the 4 partial projections per batch and adds bo.

Matmuls run as float32r (full-rate fp32 path on the PE for moving dim >= 256);
accumulation is always fp32 in PSUM. Walrus requires fp32r matmul operands to
be produced by an instruction that rounds to fp32r, so every matmul input tile
is allocated with dtype float32r and written by a DVE/ACT op (the rounding is
fused into copies we need anyway). Input transposes run in plain fp32.
"""

import numpy as np

import concourse.bacc as bacc
import concourse.mybir as mybir
import concourse.tile as tile
from concourse.bass_utils import run_bass_kernel_spmd
from concourse.masks import make_identity

F32 = mybir.dt.float32
F32R = mybir.dt.float32r

S_FULL, D_FULL, NH_PER_CORE, DH = 2048, 1024, 4, 64
N_CORES = 8
B_FULL, H_FULL = 2, 16


def build_core_program(S=S_FULL, D=D_FULL, NH=NH_PER_CORE):
    """One core's program: inputs xq/xk/xv [S,D], weight slices wq/wk/wv
    [D,NSL], wo [NSL,D], biases [NSL]; output out [S,D] (partial sum)."""
    NSL = NH * DH            # projection slice width for this core
    P = 128
    KD = D // P              # d-tiles (contraction tiles for projections)
    NT = NSL // P            # n-tiles = head-pairs
    ST = S // P              # s-tiles
    SBLK = 512 if S % 512 == 0 else S
    NB = S // SBLK           # s/i blocks
    JT = ST                  # j-tiles in attention
    JC = 2                   # j-tiles per score/exp chunk
    SS = SBLK // P           # s-subtiles per block

    nc = bacc.Bacc("TRN2", target_bir_lowering=False, debug=False)

    xq_d = nc.dram_tensor("xq", [S, D], F32, kind="ExternalInput")
    xk_d = nc.dram_tensor("xk", [S, D], F32, kind="ExternalInput")
    xv_d = nc.dram_tensor("xv", [S, D], F32, kind="ExternalInput")
    wq_d = nc.dram_tensor("wq", [D, NSL], F32, kind="ExternalInput")
    wk_d = nc.dram_tensor("wk", [D, NSL], F32, kind="ExternalInput")
    wv_d = nc.dram_tensor("wv", [D, NSL], F32, kind="ExternalInput")
    wo_d = nc.dram_tensor("wo", [NSL, D], F32, kind="ExternalInput")
    bq_d = nc.dram_tensor("bq", [NSL], F32, kind="ExternalInput")
    bk_d = nc.dram_tensor("bk", [NSL], F32, kind="ExternalInput")
    bv_d = nc.dram_tensor("bv", [NSL], F32, kind="ExternalInput")
    out_d = nc.dram_tensor("out", [S, D], F32, kind="ExternalOutput")

    with tile.TileContext(nc) as tc:
        with tc.tile_pool(name="persist", bufs=1) as pp:
            ident = pp.tile([P, P], F32)
            make_identity(nc, ident)

            # Weights: DMA fp32 staging -> rounded fp32r copies.
            wq_sb = pp.tile([P, KD, NSL], F32R)
            wk_sb = pp.tile([P, KD, NSL], F32R)
            wv_sb = pp.tile([P, KD, NSL], F32R)
            wo_sb = pp.tile([P, NT, D], F32R)
            bq_sb = pp.tile([P, NT], F32)
            nc.sync.dma_start(bq_sb, bq_d.rearrange("(t p) -> p t", p=P))
            bk_sb = pp.tile([P, NT], F32)
            nc.sync.dma_start(bk_sb, bk_d.rearrange("(t p) -> p t", p=P))
            bv_sb = pp.tile([P, NT], F32)
            nc.sync.dma_start(bv_sb, bv_d.rearrange("(t p) -> p t", p=P))

            with tc.tile_pool(name="wstage", bufs=2) as wsp:
                for w_d, w_sb, wkd, wn in (
                    (wq_d, wq_sb, KD, NSL),
                    (wk_d, wk_sb, KD, NSL),
                    (wv_d, wv_sb, KD, NSL),
                    (wo_d, wo_sb, NT, D),
                ):
                    wst = wsp.tile([P, wkd, wn], F32, tag="wst")
                    nc.sync.dma_start(
                        wst, w_d.rearrange("(t p) n -> p t n", p=P)
                    )
                    nc.vector.tensor_copy(w_sb, wst)

            # qT/o_cat are per-i-block tensors so attention / out-projection
            # dependencies stay block-granular (enables cross-phase overlap).
            qT_b = [
                pp.tile([P, NT, SBLK], F32R, name=f"qT{b}") for b in range(NB)
            ]
            kT = pp.tile([P, NT, S], F32R)
            ones_colf = pp.tile([1, DH], F32)
            nc.vector.memset(ones_colf, 1.0)
            ones_col = pp.tile([1, DH], F32R)
            nc.vector.tensor_copy(ones_col, ones_colf)
            v_sb = pp.tile([P, JT, NH, DH + 1], F32R)  # natural V + ones col
            vonesf = pp.tile([P, JT, NH, 1], F32)
            nc.vector.memset(vonesf, 1.0)
            nc.vector.tensor_copy(v_sb[:, :, :, DH : DH + 1], vonesf)
            o_b = [
                pp.tile([P, NT, SBLK], F32R, name=f"o{b}") for b in range(NB)
            ]

            # ---- Phase A: transpose inputs + projections ----
            with tc.tile_pool(name="pha", bufs=2) as pa, \
                 tc.tile_pool(name="psa", bufs=2, space="PSUM") as psa:
                plans = [
                    (xv_d, wv_sb, None, None, "v"),
                    (xk_d, wk_sb, bk_sb, None, "qk"),
                    (xq_d, wq_sb, bq_sb, qT_b, "q"),
                ]
                for x_d, w_sb, b_sb, dstT, kind in plans:
                    for blk in range(NB):
                        xn = pa.tile([P, SS, D], F32, tag="xn")
                        nc.sync.dma_start(
                            xn,
                            x_d[blk * SBLK : (blk + 1) * SBLK].rearrange(
                                "(ss p) d -> p ss d", p=P
                            ),
                        )
                        xT = pa.tile([P, KD, SBLK], F32R, tag="xT")
                        for ss in range(SS):
                            for kd in range(KD):
                                pst = psa.tile([P, P], F32, tag="pst", bufs=4)
                                nc.tensor.transpose(
                                    pst,
                                    xn[:, ss, kd * P : (kd + 1) * P],
                                    ident,
                                )
                                # split casts across DVE and the (otherwise
                                # idle in this phase) scalar engine
                                dst_sl = xT[:, kd, ss * P : (ss + 1) * P]
                                if (ss * KD + kd) % 2 == 0:
                                    nc.vector.tensor_copy(dst_sl, pst)
                                else:
                                    nc.scalar.copy(dst_sl, pst)
                        if kind in ("qk", "q"):
                            for nt in range(NT):
                                psp = psa.tile([P, SBLK], F32, tag="psp")
                                for kd in range(KD):
                                    nc.tensor.matmul(
                                        psp,
                                        lhsT=w_sb[:, kd, nt * P : (nt + 1) * P],
                                        rhs=xT[:, kd, :],
                                        start=(kd == 0),
                                        stop=(kd == KD - 1),
                                    )
                                dst = (
                                    dstT[blk][:, nt, :]
                                    if kind == "q"
                                    else kT[:, nt, blk * SBLK : (blk + 1) * SBLK]
                                )
                                nc.vector.tensor_scalar_add(
                                    dst, psp, b_sb[:, nt : nt + 1]
                                )
                        else:
                            for ss in range(SS):
                                psv = psa.tile([P, NSL], F32, tag="psv")
                                for kd in range(KD):
                                    nc.tensor.matmul(
                                        psv,
                                        lhsT=xT[:, kd, ss * P : (ss + 1) * P],
                                        rhs=wv_sb[:, kd, :],
                                        start=(kd == 0),
                                        stop=(kd == KD - 1),
                                    )
                                st = blk * SS + ss
                                nc.vector.tensor_copy(
                                    v_sb[:, st, :, 0:DH],
                                    psv.rearrange("p (h d) -> p h d", d=DH),
                                )

            # ---- Phase B: attention per i-block, per head-pair; the output
            # projection for each finished i-block is fused in as dense PE
            # filler (keeps the HAM clock warm through the ACT-paced chunks).
            with tc.tile_pool(name="phb", bufs=2) as pb, \
                 tc.tile_pool(name="psb", bufs=1, space="PSUM") as psb:
                for ib in range(NB):
                    for hp in range(NT):
                        ps_o = [
                            psb.tile([P, SBLK], F32, tag=f"ps_o{h01}",
                                     bufs=1, name=f"ps_o{h01}")
                            for h01 in range(2)
                        ]

                        def emit_exp_pv(jc, ps_s):
                            for h01 in range(2):
                                h = hp * 2 + h01
                                p_sb = pb.tile([P, JC, SBLK], F32R,
                                               tag=f"p_sb{h01}", bufs=4,
                                               name="p_sb")
                                nc.scalar.activation(
                                    p_sb, ps_s[h01],
                                    mybir.ActivationFunctionType.Exp,
                                    scale=float(1.0 / np.sqrt(DH)),
                                )
                                for jj in range(JC):
                                    jt = jc * JC + jj
                                    nc.tensor.matmul(
                                        ps_o[h01][0 : DH + 1, :],
                                        lhsT=v_sb[:, jt, h, :],
                                        rhs=p_sb[:, jj, :],
                                        start=(jt == 0),
                                        stop=(jt == JT - 1),
                                    )

                        prev = None
                        for jc in range(JT // JC):
                            ps_s = [
                                psb.tile([P, JC, SBLK], F32, tag="ps_s",
                                         bufs=3, name=f"ps_s{h01}")
                                for h01 in range(2)
                            ]
                            for jj in range(JC):
                                jt = jc * JC + jj
                                for h01 in range(2):
                                    base = h01 * DH
                                    nc.tensor.matmul(
                                        ps_s[h01][:, jj, :],
                                        lhsT=kT[base : base + DH, hp,
                                                jt * P : (jt + 1) * P],
                                        rhs=qT_b[ib][base : base + DH, hp, :],
                                        start=True,
                                        stop=True,
                                        tile_position=(base, 0),
                                    )
                            if prev is not None:
                                emit_exp_pv(*prev)
                            prev = (jc, ps_s)
                        emit_exp_pv(*prev)
                        for h01 in range(2):
                            base = h01 * DH
                            recf = pb.tile([1, SBLK], F32, tag="recf", bufs=2)
                            nc.vector.reciprocal(recf, ps_o[h01][DH : DH + 1, :])
                            rec = pb.tile([1, SBLK], F32R, tag="rec", bufs=2)
                            nc.vector.tensor_copy(rec, recf)
                            ps_b = psb.tile([P, JC, SBLK], F32, tag="ps_s",
                                            bufs=3, name="ps_b")[0:DH, 0, :]
                            nc.tensor.matmul(
                                ps_b, lhsT=ones_col, rhs=rec,
                                start=True, stop=True,
                            )
                            bc = pb.tile([DH, SBLK], F32, tag="bc", bufs=2)
                            nc.vector.tensor_copy(bc, ps_b)
                            o_slice = o_b[ib][base : base + DH, hp, :]
                            nc.vector.tensor_mul(o_slice, ps_o[h01][0:DH, :], bc)
                            nc.vector.tensor_scalar_add(
                                o_slice, o_slice,
                                bv_sb[base : base + DH, hp : hp + 1],
                            )

                    # output projection for this finished i-block
                    for st in range(ib * SS, (ib + 1) * SS):
                        for nb in range(D // SBLK):
                            pso = psb.tile([P, JC, SBLK], F32, tag="ps_s",
                                           bufs=3, name="pso")[:, 0, :]
                            for t in range(NT):
                                ss_off = (st - ib * SS) * P
                                nc.tensor.matmul(
                                    pso,
                                    lhsT=o_b[ib][:, t, ss_off : ss_off + P],
                                    rhs=wo_sb[:, t, nb * SBLK : (nb + 1) * SBLK],
                                    start=(t == 0),
                                    stop=(t == NT - 1),
                                )
                            ob = pb.tile([P, SBLK], F32, tag="ob", bufs=3)
                            nc.vector.tensor_copy(ob, pso)
                            nc.sync.dma_start(
                                out_d[st * P : (st + 1) * P,
                                      nb * SBLK : (nb + 1) * SBLK],
                                ob,
                            )

    nc.finalize()
    return nc


_NC_CACHE = {}


def _get_program(S, D, NH):
    key = (S, D, NH)
    if key not in _NC_CACHE:
        _NC_CACHE[key] = build_core_program(S, D, NH)
    return _NC_CACHE[key]


def kernel(q, k, v, Wq, bq, Wk, bk, Wv, bv, Wo, bo):
    q, k, v = (np.asarray(x, np.float32) for x in (q, k, v))
    Wq, Wk, Wv, Wo = (np.asarray(x, np.float32) for x in (Wq, Wk, Wv, Wo))
    bq, bk, bv, bo = (np.asarray(x, np.float32) for x in (bq, bk, bv, bo))
    B, S, D = q.shape
    GROUPS = N_CORES // B
    NSL = D // GROUPS

    nc = _get_program(S, D, NSL // DH)

    in_maps = []
    for c in range(N_CORES):
        b, hg = c // GROUPS, c % GROUPS
        sl = slice(hg * NSL, (hg + 1) * NSL)
        in_maps.append(
            {
                "xq": q[b],
                "xk": k[b],
                "xv": v[b],
                "wq": np.ascontiguousarray(Wq[:, sl]),
                "wk": np.ascontiguousarray(Wk[:, sl]),
                "wv": np.ascontiguousarray(Wv[:, sl]),
                "wo": np.ascontiguousarray(Wo[sl, :]),
                "bq": np.ascontiguousarray(bq[sl]),
                "bk": np.ascontiguousarray(bk[sl]),
                "bv": np.ascontiguousarray(bv[sl]),
            }
        )

    res = run_bass_kernel_spmd(nc, in_maps, list(range(N_CORES)))

    out = np.zeros((B, S, D), np.float32)
    for c in range(N_CORES):
        b = c // GROUPS
        out[b] += res.results[c]["out"]
    out += bo[None, None, :]
    return out



# revision 3
# speedup vs baseline: 1.3810x; 1.3810x over previous
"""Multi-head attention (B=2, S=2048, D=1024, H=16) on 8 Trainium2 cores.

Sharding: core c = (batch b, head-group hg) with b = c // 4, hg = c % 4.
Each core computes 4 heads of one batch element end-to-end and emits a
partial output projection; the host sums the 4 partials per batch + bo.

Key layout decisions (vs the fp32r baseline this evolved from):
  - x is transposed AND cast to bf16 on the host: device receives x^T [D,S]
    ready for the projection matmuls — no PE transposes, no PSUM round-trip,
    no fp32->fp32r rounding copies, and half the input DMA bytes.
  - All matmul operands are bf16 (1 cycle/row on the PE like fp32r, but half
    the LDWEIGHTS stream and roughly half the PE power -> less DVFS throttle).
    PSUM accumulation stays fp32.
  - Q^T/K^T projections in [dh, s] layout; scores computed transposed so the
    softmax denominator comes from a ones-column in V through the PV matmul.
  - exp on ScalarE with the 1/sqrt(dh) scale fused, bf16 out.
  - Softmax denominators for a head-pair batched into one
    reciprocal_approx_fast (custom DVE op, ~5x faster than reciprocal()).
  - Partial out-projection written as bf16 (halves output DMA; host
    accumulates in fp32).
"""

import numpy as np

import concourse.bacc as bacc
import concourse.mybir as mybir
import concourse.tile as tile
from concourse.bass_utils import run_bass_kernel_spmd

F32 = mybir.dt.float32
BF16 = mybir.dt.bfloat16

S_FULL, D_FULL, NH_PER_CORE, DH = 2048, 1024, 4, 64
N_CORES = 8
B_FULL, H_FULL = 2, 16


def build_core_program(S=S_FULL, D=D_FULL, NH=NH_PER_CORE):
    """One core's program: inputs xqt/xkt/xvt [D,S] bf16 (host-transposed),
    weight slices wq/wk/wv [D,NSL] bf16, wo [NSL,D] bf16, biases [NSL] f32;
    output out [S,D] bf16 (partial sum)."""
    NSL = NH * DH            # projection slice width for this core
    P = 128
    KD = D // P              # d-tiles (contraction tiles for projections)
    NT = NSL // P            # n-tiles = head-pairs
    SBLK = 512 if S % 512 == 0 else S
    NB = S // SBLK           # s/i blocks
    JT = S // P              # j-tiles in attention
    JC = 2                   # j-tiles per score/exp chunk
    SS = SBLK // P           # s-subtiles per block

    nc = bacc.Bacc("TRN2", target_bir_lowering=False, debug=False)

    xq_d = nc.dram_tensor("xqt", [D, S], BF16, kind="ExternalInput")
    xk_d = nc.dram_tensor("xkt", [D, S], BF16, kind="ExternalInput")
    xv_d = nc.dram_tensor("xvt", [D, S], BF16, kind="ExternalInput")
    wq_d = nc.dram_tensor("wq", [D, NSL], BF16, kind="ExternalInput")
    wk_d = nc.dram_tensor("wk", [D, NSL], BF16, kind="ExternalInput")
    wv_d = nc.dram_tensor("wv", [D, NSL], BF16, kind="ExternalInput")
    wo_d = nc.dram_tensor("wo", [NSL, D], BF16, kind="ExternalInput")
    bq_d = nc.dram_tensor("bq", [NSL], F32, kind="ExternalInput")
    bk_d = nc.dram_tensor("bk", [NSL], F32, kind="ExternalInput")
    bv_d = nc.dram_tensor("bv", [NSL], F32, kind="ExternalInput")
    out_d = nc.dram_tensor("out", [S, D], BF16, kind="ExternalOutput")

    with tile.TileContext(nc) as tc:
        with tc.tile_pool(name="persist", bufs=1) as pp:
            wq_sb = pp.tile([P, KD, NSL], BF16)
            nc.sync.dma_start(wq_sb, wq_d.rearrange("(t p) n -> p t n", p=P))
            wk_sb = pp.tile([P, KD, NSL], BF16)
            nc.sync.dma_start(wk_sb, wk_d.rearrange("(t p) n -> p t n", p=P))
            wv_sb = pp.tile([P, KD, NSL], BF16)
            nc.sync.dma_start(wv_sb, wv_d.rearrange("(t p) n -> p t n", p=P))
            wo_sb = pp.tile([P, NT, D], BF16)
            nc.sync.dma_start(wo_sb, wo_d.rearrange("(t p) n -> p t n", p=P))
            bq_sb = pp.tile([P, NT], F32)
            nc.sync.dma_start(bq_sb, bq_d.rearrange("(t p) -> p t", p=P))
            bk_sb = pp.tile([P, NT], F32)
            nc.sync.dma_start(bk_sb, bk_d.rearrange("(t p) -> p t", p=P))
            bv_sb = pp.tile([P, NT], F32)
            nc.sync.dma_start(bv_sb, bv_d.rearrange("(t p) -> p t", p=P))

            # qT/o are per-i-block tensors so attention / out-projection
            # dependencies stay block-granular (enables cross-phase overlap).
            qT_b = [
                pp.tile([P, NT, SBLK], BF16, name=f"qT{b}") for b in range(NB)
            ]
            kT = pp.tile([P, NT, S], BF16)
            ones_col = pp.tile([1, DH], BF16)
            nc.vector.memset(ones_col, 1.0)
            v_sb = pp.tile([P, JT, NH, DH + 1], BF16)  # natural V + ones col
            nc.vector.memset(v_sb[:, :, :, DH : DH + 1], 1.0)
            o_b = [
                pp.tile([P, NT, SBLK], BF16, name=f"o{b}") for b in range(NB)
            ]

            # ---- Phase A: projections straight from host-transposed x ----
            with tc.tile_pool(name="pha", bufs=2) as pa, \
                 tc.tile_pool(name="psa", bufs=2, space="PSUM") as psa:
                plans = [
                    (xv_d, wv_sb, None, None, "v"),
                    (xk_d, wk_sb, bk_sb, None, "qk"),
                    (xq_d, wq_sb, bq_sb, qT_b, "q"),
                ]
                for x_d, w_sb, b_sb, dstT, kind in plans:
                    for blk in range(NB):
                        xt = pa.tile([P, KD, SBLK], BF16, tag="xt")
                        nc.sync.dma_start(
                            xt,
                            x_d.rearrange("(kd p) s -> p kd s", p=P)[
                                :, :, blk * SBLK : (blk + 1) * SBLK
                            ],
                        )
                        if kind in ("qk", "q"):
                            for nt in range(NT):
                                psp = psa.tile([P, SBLK], F32, tag="psp")
                                for kd in range(KD):
                                    nc.tensor.matmul(
                                        psp,
                                        lhsT=w_sb[:, kd, nt * P : (nt + 1) * P],
                                        rhs=xt[:, kd, :],
                                        start=(kd == 0),
                                        stop=(kd == KD - 1),
                                    )
                                dst = (
                                    dstT[blk][:, nt, :]
                                    if kind == "q"
                                    else kT[:, nt, blk * SBLK : (blk + 1) * SBLK]
                                )
                                nc.vector.tensor_scalar_add(
                                    dst, psp, b_sb[:, nt : nt + 1]
                                )
                        else:
                            for ss in range(SS):
                                psv = psa.tile([P, NSL], F32, tag="psv")
                                for kd in range(KD):
                                    nc.tensor.matmul(
                                        psv,
                                        lhsT=xt[:, kd, ss * P : (ss + 1) * P],
                                        rhs=wv_sb[:, kd, :],
                                        start=(kd == 0),
                                        stop=(kd == KD - 1),
                                    )
                                st = blk * SS + ss
                                nc.vector.tensor_copy(
                                    v_sb[:, st, :, 0:DH],
                                    psv.rearrange("p (h d) -> p h d", d=DH),
                                )

            # ---- Phase B: attention per i-block, per head-pair; the output
            # projection for each finished i-block is fused in as dense PE
            # filler.
            with tc.tile_pool(name="phb", bufs=2) as pb, \
                 tc.tile_pool(name="psb", bufs=1, space="PSUM") as psb:
                for ib in range(NB):
                    for hp in range(NT):
                        ps_o = [
                            psb.tile([P, SBLK], F32, tag=f"ps_o{h01}",
                                     bufs=1, name=f"ps_o{h01}")
                            for h01 in range(2)
                        ]

                        def emit_exp_pv(jc, ps_s):
                            for h01 in range(2):
                                h = hp * 2 + h01
                                p_sb = pb.tile([P, JC, SBLK], BF16,
                                               tag=f"p_sb{h01}", bufs=4,
                                               name="p_sb")
                                nc.scalar.activation(
                                    p_sb, ps_s[h01],
                                    mybir.ActivationFunctionType.Exp,
                                    scale=float(1.0 / np.sqrt(DH)),
                                )
                                for jj in range(JC):
                                    jt = jc * JC + jj
                                    nc.tensor.matmul(
                                        ps_o[h01][0 : DH + 1, :],
                                        lhsT=v_sb[:, jt, h, :],
                                        rhs=p_sb[:, jj, :],
                                        start=(jt == 0),
                                        stop=(jt == JT - 1),
                                    )

                        prev = None
                        for jc in range(JT // JC):
                            ps_s = [
                                psb.tile([P, JC, SBLK], F32, tag="ps_s",
                                         bufs=3, name=f"ps_s{h01}")
                                for h01 in range(2)
                            ]
                            for jj in range(JC):
                                jt = jc * JC + jj
                                for h01 in range(2):
                                    base = h01 * DH
                                    nc.tensor.matmul(
                                        ps_s[h01][:, jj, :],
                                        lhsT=kT[base : base + DH, hp,
                                                jt * P : (jt + 1) * P],
                                        rhs=qT_b[ib][base : base + DH, hp, :],
                                        start=True,
                                        stop=True,
                                        tile_position=(base, 0),
                                    )
                            if prev is not None:
                                emit_exp_pv(*prev)
                            prev = (jc, ps_s)
                        emit_exp_pv(*prev)

                        # normalize: batch both heads' denominators into one
                        # fast reciprocal (single partition, heads side by
                        # side in the free dim so the broadcast matmul rhs
                        # stays at base partition 0), then broadcast via a
                        # ones-column matmul and scale the PV accumulators.
                        den = pb.tile([1, 2, SBLK], F32, tag="den", bufs=2)
                        for h01 in range(2):
                            nc.vector.tensor_copy(
                                den[0:1, h01, :],
                                ps_o[h01][DH : DH + 1, :],
                            )
                        rec = pb.tile([1, 2, SBLK], F32, tag="rec", bufs=2)
                        nc.vector.reciprocal_approx_fast(rec, den)
                        recb = pb.tile([1, 2, SBLK], BF16, tag="recb", bufs=2)
                        nc.vector.tensor_copy(recb, rec)
                        for h01 in range(2):
                            base = h01 * DH
                            ps_b = psb.tile([P, JC, SBLK], F32, tag="ps_s",
                                            bufs=3, name="ps_b")[0:DH, 0, :]
                            nc.tensor.matmul(
                                ps_b, lhsT=ones_col,
                                rhs=recb[0:1, h01, :],
                                start=True, stop=True,
                            )
                            bc = pb.tile([DH, SBLK], F32, tag="bc", bufs=2)
                            nc.vector.tensor_copy(bc, ps_b)
                            o_slice = o_b[ib][base : base + DH, hp, :]
                            nc.vector.tensor_mul(o_slice, ps_o[h01][0:DH, :], bc)
                            nc.vector.tensor_scalar_add(
                                o_slice, o_slice,
                                bv_sb[base : base + DH, hp : hp + 1],
                            )

                    # output projection for this finished i-block
                    for st in range(ib * SS, (ib + 1) * SS):
                        for nb in range(D // SBLK):
                            pso = psb.tile([P, JC, SBLK], F32, tag="ps_s",
                                           bufs=3, name="pso")[:, 0, :]
                            for t in range(NT):
                                ss_off = (st - ib * SS) * P
                                nc.tensor.matmul(
                                    pso,
                                    lhsT=o_b[ib][:, t, ss_off : ss_off + P],
                                    rhs=wo_sb[:, t, nb * SBLK : (nb + 1) * SBLK],
                                    start=(t == 0),
                                    stop=(t == NT - 1),
                                )
                            ob = pb.tile([P, SBLK], BF16, tag="ob", bufs=3)
                            nc.vector.tensor_copy(ob, pso)
                            nc.sync.dma_start(
                                out_d[st * P : (st + 1) * P,
                                      nb * SBLK : (nb + 1) * SBLK],
                                ob,
                            )

    nc.finalize()
    return nc


_NC_CACHE = {}


def _get_program(S, D, NH):
    key = (S, D, NH)
    if key not in _NC_CACHE:
        _NC_CACHE[key] = build_core_program(S, D, NH)
    return _NC_CACHE[key]


def make_in_maps(q, k, v, Wq, bq, Wk, bk, Wv, bv, Wo):
    """Host-side sharding: transpose+cast x to bf16 once per batch element,
    slice weights per head-group. Returns the per-core input maps."""
    import ml_dtypes

    bf16 = ml_dtypes.bfloat16
    B, S, D = q.shape
    GROUPS = N_CORES // B
    NSL = D // GROUPS

    xqt = [np.asarray(q[b], np.float32).T.astype(bf16) for b in range(B)]
    xkt = [np.asarray(k[b], np.float32).T.astype(bf16) for b in range(B)]
    xvt = [np.asarray(v[b], np.float32).T.astype(bf16) for b in range(B)]
    Wq, Wk, Wv, Wo = (np.asarray(x, np.float32) for x in (Wq, Wk, Wv, Wo))
    bq, bk, bv = (np.asarray(x, np.float32) for x in (bq, bk, bv))

    in_maps = []
    for c in range(N_CORES):
        b, hg = c // GROUPS, c % GROUPS
        sl = slice(hg * NSL, (hg + 1) * NSL)
        in_maps.append(
            {
                "xqt": xqt[b],
                "xkt": xkt[b],
                "xvt": xvt[b],
                "wq": Wq[:, sl].astype(bf16),
                "wk": Wk[:, sl].astype(bf16),
                "wv": Wv[:, sl].astype(bf16),
                "wo": np.ascontiguousarray(Wo[sl, :]).astype(bf16),
                "bq": np.ascontiguousarray(bq[sl]),
                "bk": np.ascontiguousarray(bk[sl]),
                "bv": np.ascontiguousarray(bv[sl]),
            }
        )
    return in_maps


def kernel(q, k, v, Wq, bq, Wk, bk, Wv, bv, Wo, bo):
    B, S, D = q.shape
    GROUPS = N_CORES // B
    NSL = D // GROUPS

    nc = _get_program(S, D, NSL // DH)
    in_maps = make_in_maps(q, k, v, Wq, bq, Wk, bk, Wv, bv, Wo)
    res = run_bass_kernel_spmd(nc, in_maps, list(range(N_CORES)))

    out = np.zeros((B, S, D), np.float32)
    for c in range(N_CORES):
        b = c // GROUPS
        out[b] += np.asarray(res.results[c]["out"], np.float32)
    out += np.asarray(bo, np.float32)[None, None, :]
    return out


# revision 5
# speedup vs baseline: 1.6609x; 1.2026x over previous
"""Multi-head attention (B=2, S=2048, D=1024, H=16) on 8 Trainium2 cores.

Sharding: core c = (batch b, head-group hg) with b = c // 4, hg = c % 4.
Each core computes 4 heads of one batch element end-to-end and emits a
partial output projection; the host sums the 4 partials per batch and adds
bo + bv @ Wo (the V-bias commutes through softmax-normalized attention, so
it is applied host-side).

Performance-critical structure (v3):
  - x is transposed AND cast to bf16 on the host: device receives x^T [D,S]
    ready for the projection matmuls — no PE transposes, half the input DMA.
  - All matmul operands bf16 (1 cycle/row, half the LDWEIGHTS stream, less
    PE power -> less DVFS throttle). PSUM accumulation fp32.
  - The PE instruction stream has NO data-dependent stalls: the softmax
    normalize chain runs entirely on DVE+GPSIMD (reciprocal_approx_fast +
    partition_broadcast), PV accumulators are evacuated to SBUF right after
    the accumulation group closes, exp->PV is pipelined 2 chunks deep, and
    each block's output projection is deferred and drip-fed into the next
    block's attention as PE filler. A stall-free stream lets the PE DVFS
    ramp hold its top p-state (2.4 GHz vs the ~1.4 GHz it averages when the
    stream hiccups every few microseconds).
  - exp on ScalarE (the true lower bound of this kernel alongside the PE:
    16.8M exps/core at 1.2 GHz x 128 lanes), bf16 out, 1/sqrt(dh) fused.
"""

import numpy as np

import concourse.bacc as bacc
import concourse.mybir as mybir
import concourse.tile as tile
from concourse.bass_utils import run_bass_kernel_spmd

F32 = mybir.dt.float32
BF16 = mybir.dt.bfloat16

S_FULL, D_FULL, NH_PER_CORE, DH = 2048, 1024, 4, 64
N_CORES = 8
B_FULL, H_FULL = 2, 16


def build_core_program(S=S_FULL, D=D_FULL, NH=NH_PER_CORE):
    """One core's program: inputs xqt/xkt/xvt [D,S] bf16 (host-transposed),
    weight slices wq/wk/wv [D,NSL] bf16, wo [NSL,D] bf16, biases bq/bk [NSL]
    f32; output out [S,D] bf16 (partial sum, no biases)."""
    NSL = NH * DH            # projection slice width for this core
    P = 128
    KD = D // P              # d-tiles (contraction tiles for projections)
    NT = NSL // P            # n-tiles = head-pairs
    SBLK = 512 if S % 512 == 0 else S
    NB = S // SBLK           # s/i blocks
    JT = S // P              # j-tiles in attention
    JC = 2                   # j-tiles per score/exp chunk
    NJC = JT // JC           # chunks per (i-block, head-pair)
    SS = SBLK // P           # s-subtiles per block

    nc = bacc.Bacc("TRN2", target_bir_lowering=False, debug=False)

    xq_d = nc.dram_tensor("xqt", [D, S], BF16, kind="ExternalInput")
    xk_d = nc.dram_tensor("xkt", [D, S], BF16, kind="ExternalInput")
    xv_d = nc.dram_tensor("xvt", [D, S], BF16, kind="ExternalInput")
    wq_d = nc.dram_tensor("wq", [D, NSL], BF16, kind="ExternalInput")
    wk_d = nc.dram_tensor("wk", [D, NSL], BF16, kind="ExternalInput")
    wv_d = nc.dram_tensor("wv", [D, NSL], BF16, kind="ExternalInput")
    wo_d = nc.dram_tensor("wo", [NSL, D], BF16, kind="ExternalInput")
    bq_d = nc.dram_tensor("bq", [NSL], F32, kind="ExternalInput")
    bk_d = nc.dram_tensor("bk", [NSL], F32, kind="ExternalInput")
    out_d = nc.dram_tensor("out", [S, D], BF16, kind="ExternalOutput")

    with tile.TileContext(nc) as tc:
        with tc.tile_pool(name="persist", bufs=1) as pp:
            # DMA order matters: the v-projection only needs wv + the first
            # xvt block, so those go first; everything else queues behind.
            wv_sb = pp.tile([P, KD, NSL], BF16)
            nc.sync.dma_start(wv_sb, wv_d.rearrange("(t p) n -> p t n", p=P))
            wk_sb = pp.tile([P, KD, NSL], BF16)
            wq_sb = pp.tile([P, KD, NSL], BF16)
            wo_sb = pp.tile([P, NT, D], BF16)
            bq_sb = pp.tile([P, NT], F32)
            bk_sb = pp.tile([P, NT], F32)

            qT_b = [
                pp.tile([P, NT, SBLK], BF16, name=f"qT{b}") for b in range(NB)
            ]
            kT = pp.tile([P, NT, S], BF16)
            v_sb = pp.tile([P, JT, NH, DH + 1], BF16)  # natural V + ones col
            nc.vector.memset(v_sb[:, :, :, DH : DH + 1], 1.0)
            o_b = [
                pp.tile([P, NT, SBLK], BF16, name=f"o{b}") for b in range(NB)
            ]

            # ---- Phase A: projections straight from host-transposed x ----
            with tc.tile_pool(name="pha", bufs=2) as pa, \
                 tc.tile_pool(name="psa", bufs=2, space="PSUM") as psa:
                plans = [
                    (xv_d, wv_sb, None, None, "v"),
                    (xk_d, wk_sb, bk_sb, None, "qk"),
                    (xq_d, wq_sb, bq_sb, qT_b, "q"),
                ]
                for x_d, w_sb, b_sb, dstT, kind in plans:
                    for blk in range(NB):
                        xt = pa.tile([P, KD, SBLK], BF16, tag="xt")
                        nc.sync.dma_start(
                            xt,
                            x_d.rearrange("(kd p) s -> p kd s", p=P)[
                                :, :, blk * SBLK : (blk + 1) * SBLK
                            ],
                        )
                        if kind == "v" and blk == 0:
                            # remaining weights ride behind the first x block
                            nc.sync.dma_start(
                                wk_sb,
                                wk_d.rearrange("(t p) n -> p t n", p=P),
                            )
                            nc.sync.dma_start(
                                wq_sb,
                                wq_d.rearrange("(t p) n -> p t n", p=P),
                            )
                            nc.sync.dma_start(
                                wo_sb,
                                wo_d.rearrange("(t p) n -> p t n", p=P),
                            )
                            nc.sync.dma_start(
                                bq_sb, bq_d.rearrange("(t p) -> p t", p=P)
                            )
                            nc.sync.dma_start(
                                bk_sb, bk_d.rearrange("(t p) -> p t", p=P)
                            )
                        if kind in ("qk", "q"):
                            for nt in range(NT):
                                psp = psa.tile([P, SBLK], F32, tag="psp")
                                for kd in range(KD):
                                    nc.tensor.matmul(
                                        psp,
                                        lhsT=w_sb[:, kd, nt * P : (nt + 1) * P],
                                        rhs=xt[:, kd, :],
                                        start=(kd == 0),
                                        stop=(kd == KD - 1),
                                    )
                                dst = (
                                    dstT[blk][:, nt, :]
                                    if kind == "q"
                                    else kT[:, nt, blk * SBLK : (blk + 1) * SBLK]
                                )
                                nc.vector.tensor_scalar_add(
                                    dst, psp, b_sb[:, nt : nt + 1]
                                )
                        else:
                            for ss in range(SS):
                                psv = psa.tile([P, NSL], F32, tag="psv")
                                for kd in range(KD):
                                    nc.tensor.matmul(
                                        psv,
                                        lhsT=xt[:, kd, ss * P : (ss + 1) * P],
                                        rhs=wv_sb[:, kd, :],
                                        start=(kd == 0),
                                        stop=(kd == KD - 1),
                                    )
                                st = blk * SS + ss
                                nc.vector.tensor_copy(
                                    v_sb[:, st, :, 0:DH],
                                    psv.rearrange("p (h d) -> p h d", d=DH),
                                )

            # ---- Phase B: attention per (i-block, head-pair). The PE stream
            # is kept stall-free: normalize runs off-PE, out-projections are
            # deferred and drip-fed as filler into later pairs.
            with tc.tile_pool(name="phb", bufs=2) as pb, \
                 tc.tile_pool(name="psb", bufs=1, space="PSUM") as psb:

                chunk_ctr = 0           # global jc-step counter
                pending_op = []         # (ready_at_chunk, ib, st, nb)

                def emit_outproj_unit(ib, st, nb):
                    pso = psb.tile([P, JC, SBLK], F32, tag="ps_s",
                                   bufs=3, name="pso")[:, 0, :]
                    for t in range(NT):
                        ss_off = (st - ib * SS) * P
                        nc.tensor.matmul(
                            pso,
                            lhsT=o_b[ib][:, t, ss_off : ss_off + P],
                            rhs=wo_sb[:, t, nb * SBLK : (nb + 1) * SBLK],
                            start=(t == 0),
                            stop=(t == NT - 1),
                        )
                    ob = pb.tile([P, SBLK], BF16, tag="ob", bufs=3)
                    nc.vector.tensor_copy(ob, pso)
                    nc.sync.dma_start(
                        out_d[st * P : (st + 1) * P,
                              nb * SBLK : (nb + 1) * SBLK],
                        ob,
                    )

                def drain_outproj(limit):
                    n = 0
                    while pending_op and pending_op[0][0] <= chunk_ctr \
                            and n < limit:
                        _, ib_, st_, nb_ = pending_op.pop(0)
                        emit_outproj_unit(ib_, st_, nb_)
                        n += 1

                for ib in range(NB):
                    for hp in range(NT):
                        ps_o = [
                            psb.tile([P, SBLK], F32, tag=f"ps_o{h01}",
                                     bufs=1, name=f"ps_o{h01}")
                            for h01 in range(2)
                        ]

                        def emit_exp_pv(jt, ps_s):
                            # one exp covers both heads of this j-tile
                            p_sb = pb.tile([P, 2, SBLK], BF16,
                                           tag="p_sb", bufs=4, name="p_sb")
                            nc.scalar.activation(
                                p_sb, ps_s,
                                mybir.ActivationFunctionType.Exp,
                                scale=float(1.0 / np.sqrt(DH)),
                            )
                            for h01 in range(2):
                                h = hp * 2 + h01
                                nc.tensor.matmul(
                                    ps_o[h01][0 : DH + 1, :],
                                    lhsT=v_sb[:, jt, h, :],
                                    rhs=p_sb[:, h01, :],
                                    start=(jt == 0),
                                    stop=(jt == JT - 1),
                                )

                        # chunk = one j-tile, both heads packed into a single
                        # 2-bank PSUM tile -> three chunks rotate in PSUM and
                        # the exp->PV pipeline (2 deep) never starves the PE.
                        pipe = []
                        for jt in range(JT):
                            ps_s = psb.tile([P, 2, SBLK], F32, tag="ps_s",
                                            bufs=3, name="ps_s")
                            for h01 in range(2):
                                base = h01 * DH
                                nc.tensor.matmul(
                                    ps_s[:, h01, :],
                                    lhsT=kT[base : base + DH, hp,
                                            jt * P : (jt + 1) * P],
                                    rhs=qT_b[ib][base : base + DH, hp, :],
                                    start=True,
                                    stop=True,
                                    tile_position=(base, 0),
                                )
                            chunk_ctr += 1
                            drain_outproj(limit=1)
                            if len(pipe) == 2:
                                emit_exp_pv(*pipe.pop(0))
                            pipe.append((jt, ps_s))
                        for item in pipe:
                            emit_exp_pv(*item)
                        pipe.clear()

                        # Evacuate PV accumulators (frees the PSUM banks for
                        # the next pair), then normalize entirely off the PE:
                        # fast-reciprocal on DVE, partition broadcast on
                        # GPSIMD, scale on DVE. The denominator row is staged
                        # through a partition-0 tile first: the custom-DVE
                        # reciprocal misreads partition-offset inputs.
                        o_un = pb.tile([DH + 1, 2, SBLK], F32, tag="o_un",
                                       bufs=2)
                        for h01 in range(2):
                            nc.vector.tensor_copy(
                                o_un[:, h01, :], ps_o[h01][0 : DH + 1, :]
                            )
                        den0 = pb.tile([1, 2, SBLK], F32, tag="den0", bufs=2)
                        nc.vector.tensor_copy(den0, o_un[DH : DH + 1, :, :])
                        rec = pb.tile([1, 2, SBLK], F32, tag="rec", bufs=2)
                        nc.vector.reciprocal_approx_fast(rec, den0)
                        bc = pb.tile([DH, 2, SBLK], F32, tag="bc", bufs=2)
                        nc.gpsimd.partition_broadcast(bc, rec)
                        for h01 in range(2):
                            base = h01 * DH
                            o_slice = o_b[ib][base : base + DH, hp, :]
                            nc.vector.tensor_mul(
                                o_slice, o_un[0:DH, h01, :], bc[:, h01, :]
                            )

                    # defer this block's output projection into later pairs
                    for u in range(2 * SS * (D // SBLK) // NT):
                        st = ib * SS + u // (D // SBLK)
                        nb = u % (D // SBLK)
                        pending_op.append((chunk_ctr + 3, ib, st, nb))

                while pending_op:
                    _, ib_, st_, nb_ = pending_op.pop(0)
                    emit_outproj_unit(ib_, st_, nb_)

    nc.finalize()
    return nc


_NC_CACHE = {}


def _get_program(S, D, NH):
    key = (S, D, NH)
    if key not in _NC_CACHE:
        _NC_CACHE[key] = build_core_program(S, D, NH)
    return _NC_CACHE[key]


def make_in_maps(q, k, v, Wq, bq, Wk, bk, Wv, bv, Wo):
    """Host-side sharding: transpose+cast x to bf16 once per batch element,
    slice weights per head-group. Returns the per-core input maps."""
    import ml_dtypes

    bf16 = ml_dtypes.bfloat16
    B, S, D = q.shape
    GROUPS = N_CORES // B
    NSL = D // GROUPS

    xqt = [np.asarray(q[b], np.float32).T.astype(bf16) for b in range(B)]
    xkt = [np.asarray(k[b], np.float32).T.astype(bf16) for b in range(B)]
    xvt = [np.asarray(v[b], np.float32).T.astype(bf16) for b in range(B)]
    Wq, Wk, Wv, Wo = (np.asarray(x, np.float32) for x in (Wq, Wk, Wv, Wo))
    bq, bk = (np.asarray(x, np.float32) for x in (bq, bk))

    in_maps = []
    for c in range(N_CORES):
        b, hg = c // GROUPS, c % GROUPS
        sl = slice(hg * NSL, (hg + 1) * NSL)
        in_maps.append(
            {
                "xqt": xqt[b],
                "xkt": xkt[b],
                "xvt": xvt[b],
                "wq": Wq[:, sl].astype(bf16),
                "wk": Wk[:, sl].astype(bf16),
                "wv": Wv[:, sl].astype(bf16),
                "wo": np.ascontiguousarray(Wo[sl, :]).astype(bf16),
                "bq": np.ascontiguousarray(bq[sl]),
                "bk": np.ascontiguousarray(bk[sl]),
            }
        )
    return in_maps


def kernel(q, k, v, Wq, bq, Wk, bk, Wv, bv, Wo, bo):
    B, S, D = q.shape
    GROUPS = N_CORES // B
    NSL = D // GROUPS

    nc = _get_program(S, D, NSL // DH)
    in_maps = make_in_maps(q, k, v, Wq, bq, Wk, bk, Wv, bv, Wo)
    res = run_bass_kernel_spmd(nc, in_maps, list(range(N_CORES)))

    out = np.zeros((B, S, D), np.float32)
    for c in range(N_CORES):
        b = c // GROUPS
        out[b] += np.asarray(res.results[c]["out"], np.float32)
    # bv commutes through the softmax (rows sum to 1): P @ (V + bv) @ Wo =
    # P @ V @ Wo + bv @ Wo. Apply bv@Wo + bo host-side.
    bias = np.asarray(bv, np.float32) @ np.asarray(Wo, np.float32)
    bias += np.asarray(bo, np.float32)
    out += bias[None, None, :]
    return out


# revision 7
# speedup vs baseline: 1.7567x; 1.0577x over previous
"""Multi-head attention (B=2, S=2048, D=1024, H=16) on 8 Trainium2 cores.

Sharding: core c = (batch b, head-group hg) with b = c // 4, hg = c % 4.
Each core computes 4 heads of one batch element end-to-end and emits a
partial output projection; the host sums the 4 partials per batch and adds
bo + bv @ Wo (the V-bias commutes through softmax-normalized attention).

Performance-critical structure (v5):
  - x is transposed AND cast to bf16 on the host; all matmul operands bf16;
    PSUM accumulation fp32.
  - Projection accumulation chains alternate between two PSUM banks:
    back-to-back accumulating matmuls into the SAME bank serialize their
    array fill/drain (~375 ns per 512-cycle matmul vs ~213 ns pipelined).
  - Phase B is paced by ScalarE exp (16.8M exps/core is the hard floor, ~1.1
    us per [128,1024] tile). Everything else is arranged to never make ACT
    wait: score chunks are per-j-tile with both heads packed in one 2-bank
    PSUM tile (3 chunks rotate in 6 banks; PV accumulators pin the other 2),
    the exp->PV pipeline runs 2 chunks deep through a GLOBAL chunk stream
    that crosses head-pair/block boundaries without flushing, and the
    normalize chain runs entirely on DVE+GPSIMD.
  - q-projections for blocks 1..3 and each finished block's out-projection
    are drip-fed into the phase-B chunk stream as PE filler.
  - The LAST block's out-projection is split by head-pair: the t=0 half runs
    as soon as its normalize lands (partial written to `out`), the t=1 half
    goes to a separate small `out1` tensor summed on the host — this turns a
    ~20 us serial tail into ~6 us.
"""

import numpy as np

import concourse.bacc as bacc
import concourse.mybir as mybir
import concourse.tile as tile
from concourse.bass_utils import run_bass_kernel_spmd

F32 = mybir.dt.float32
BF16 = mybir.dt.bfloat16

S_FULL, D_FULL, NH_PER_CORE, DH = 2048, 1024, 4, 64
N_CORES = 8
B_FULL, H_FULL = 2, 16


def build_core_program(S=S_FULL, D=D_FULL, NH=NH_PER_CORE):
    """One core's program: inputs xqt/xkt/xvt [D,S] bf16 (host-transposed),
    weight slices wq/wk/wv [D,NSL] bf16, wo [NSL,D] bf16, biases bq/bk [NSL]
    f32; outputs out [S,D] bf16 (partial, last block t=0-only) and out1
    [SBLK,D] bf16 (last block t=1 partial)."""
    NSL = NH * DH
    P = 128
    KD = D // P
    NT = NSL // P            # head-pairs
    SBLK = 512 if S % 512 == 0 else S
    NB = S // SBLK
    JT = S // P
    SS = SBLK // P

    nc = bacc.Bacc("TRN2", target_bir_lowering=False, debug=False)

    xq_d = nc.dram_tensor("xqt", [D, S], BF16, kind="ExternalInput")
    xk_d = nc.dram_tensor("xkt", [D, S], BF16, kind="ExternalInput")
    xv_d = nc.dram_tensor("xvt", [D, S], BF16, kind="ExternalInput")
    wq_d = nc.dram_tensor("wq", [D, NSL], BF16, kind="ExternalInput")
    wk_d = nc.dram_tensor("wk", [D, NSL], BF16, kind="ExternalInput")
    wv_d = nc.dram_tensor("wv", [D, NSL], BF16, kind="ExternalInput")
    wo_d = nc.dram_tensor("wo", [NSL, D], BF16, kind="ExternalInput")
    bq_d = nc.dram_tensor("bq", [NSL], F32, kind="ExternalInput")
    bk_d = nc.dram_tensor("bk", [NSL], F32, kind="ExternalInput")
    out_d = nc.dram_tensor("out", [S, D], BF16, kind="ExternalOutput")
    out1_d = nc.dram_tensor("out1", [SBLK, D], BF16, kind="ExternalOutput")

    with tile.TileContext(nc) as tc:
        with tc.tile_pool(name="persist", bufs=1) as pp:
            wv_sb = pp.tile([P, KD, NSL], BF16)
            nc.sync.dma_start(wv_sb, wv_d.rearrange("(t p) n -> p t n", p=P))
            wk_sb = pp.tile([P, KD, NSL], BF16)
            wq_sb = pp.tile([P, KD, NSL], BF16)
            wo_sb = pp.tile([P, NT, D], BF16)
            bq_sb = pp.tile([P, NT], F32)
            bk_sb = pp.tile([P, NT], F32)

            qT_b = [
                pp.tile([P, NT, SBLK], BF16, name=f"qT{b}") for b in range(NB)
            ]
            kT = pp.tile([P, NT, S], BF16)
            v_sb = pp.tile([P, JT, NH, DH + 1], BF16)
            nc.vector.memset(v_sb[:, :, :, DH : DH + 1], 1.0)
            o_b = [
                pp.tile([P, NT, SBLK], BF16, name=f"o{b}") for b in range(NB)
            ]

            xre = xq_d.rearrange("(kd p) s -> p kd s", p=P)
            xke = xk_d.rearrange("(kd p) s -> p kd s", p=P)
            xve = xv_d.rearrange("(kd p) s -> p kd s", p=P)

            with tc.tile_pool(name="stage", bufs=2) as pa:
                # ---- Phase A: v + k projections for all blocks, q block 0.
                with tc.tile_pool(name="psa", bufs=2, space="PSUM") as psa:
                    for kind, x_e in (("v", xve), ("k", xke), ("q0", xre)):
                        blks = range(NB) if kind != "q0" else range(1)
                        for blk in blks:
                            xt = pa.tile([P, KD, SBLK], BF16, tag="xt")
                            nc.sync.dma_start(
                                xt,
                                x_e[:, :, blk * SBLK : (blk + 1) * SBLK],
                            )
                            if kind == "v" and blk == 0:
                                nc.sync.dma_start(
                                    wk_sb,
                                    wk_d.rearrange("(t p) n -> p t n", p=P),
                                )
                                nc.sync.dma_start(
                                    wq_sb,
                                    wq_d.rearrange("(t p) n -> p t n", p=P),
                                )
                                nc.sync.dma_start(
                                    wo_sb,
                                    wo_d.rearrange("(t p) n -> p t n", p=P),
                                )
                                nc.sync.dma_start(
                                    bq_sb, bq_d.rearrange("(t p) -> p t", p=P)
                                )
                                nc.sync.dma_start(
                                    bk_sb, bk_d.rearrange("(t p) -> p t", p=P)
                                )
                            if kind == "v":
                                # alternate the two psv banks so accumulation
                                # chains pipeline on the PE
                                for sp in range(SS // 2):
                                    psv = [
                                        psa.tile([P, NSL], F32, tag="psv",
                                                 name=f"psv{i}")
                                        for i in range(2)
                                    ]
                                    for kd in range(KD):
                                        for i in range(2):
                                            ss = sp * 2 + i
                                            nc.tensor.matmul(
                                                psv[i],
                                                lhsT=xt[:, kd,
                                                        ss * P : (ss + 1) * P],
                                                rhs=wv_sb[:, kd, :],
                                                start=(kd == 0),
                                                stop=(kd == KD - 1),
                                            )
                                    for i in range(2):
                                        st = blk * SS + sp * 2 + i
                                        nc.vector.tensor_copy(
                                            v_sb[:, st, :, 0:DH],
                                            psv[i].rearrange(
                                                "p (h d) -> p h d", d=DH
                                            ),
                                        )
                            else:
                                w_sb = wk_sb if kind == "k" else wq_sb
                                b_sb = bk_sb if kind == "k" else bq_sb
                                psp = [
                                    psa.tile([P, SBLK], F32, tag="psp",
                                             name=f"psp{i}")
                                    for i in range(NT)
                                ]
                                for kd in range(KD):
                                    for nt in range(NT):
                                        nc.tensor.matmul(
                                            psp[nt],
                                            lhsT=w_sb[:, kd,
                                                      nt * P : (nt + 1) * P],
                                            rhs=xt[:, kd, :],
                                            start=(kd == 0),
                                            stop=(kd == KD - 1),
                                        )
                                for nt in range(NT):
                                    dst = (
                                        qT_b[blk][:, nt, :]
                                        if kind == "q0"
                                        else kT[:, nt,
                                                blk * SBLK : (blk + 1) * SBLK]
                                    )
                                    nc.vector.tensor_scalar_add(
                                        dst, psp[nt], b_sb[:, nt : nt + 1]
                                    )

                # xq blocks 1..3 staged for the phase-B q-projection drip
                xq_drip = []
                for blk in range(1, NB):
                    xt = pa.tile([P, KD, SBLK], BF16, tag="xqd")
                    nc.sync.dma_start(
                        xt, xre[:, :, blk * SBLK : (blk + 1) * SBLK]
                    )
                    xq_drip.append((blk, xt))

                # ---- Phase B: global chunk stream over (ib, hp, jt) ----
                with tc.tile_pool(name="phb", bufs=2) as pb, \
                     tc.tile_pool(name="psb", bufs=1, space="PSUM") as psb:

                    state = {"chunk": 0}
                    drip = []      # (ready_chunk, emit_fn)
                    ps_o_of = {}   # pair -> [ps_o0, ps_o1]

                    def get_ps_o(pair):
                        if pair not in ps_o_of:
                            ps_o_of[pair] = [
                                psb.tile([P, SBLK], F32, tag=f"ps_o{h01}",
                                         bufs=1, name=f"ps_o{h01}")
                                for h01 in range(2)
                            ]
                        return ps_o_of[pair]

                    def emit_exp_pv(pair, jt, ps_s):
                        ib, hp = pair
                        ps_o = get_ps_o(pair)
                        p_sb = pb.tile([P, 2, SBLK], BF16, tag="p_sb",
                                       bufs=4, name="p_sb")
                        nc.scalar.activation(
                            p_sb, ps_s,
                            mybir.ActivationFunctionType.Exp,
                            scale=float(1.0 / np.sqrt(DH)),
                        )
                        for h01 in range(2):
                            h = hp * 2 + h01
                            nc.tensor.matmul(
                                ps_o[h01][0 : DH + 1, :],
                                lhsT=v_sb[:, jt, h, :],
                                rhs=p_sb[:, h01, :],
                                start=(jt == 0),
                                stop=(jt == JT - 1),
                            )
                        if jt == JT - 1:
                            emit_normalize(pair)

                    def emit_normalize(pair):
                        ib, hp = pair
                        ps_o = ps_o_of[pair]
                        o_un = pb.tile([DH + 1, 2, SBLK], F32, tag="o_un",
                                       bufs=2)
                        for h01 in range(2):
                            nc.vector.tensor_copy(
                                o_un[:, h01, :], ps_o[h01][0 : DH + 1, :]
                            )
                        den0 = pb.tile([1, 2, SBLK], F32, tag="den0", bufs=2)
                        nc.vector.tensor_copy(den0, o_un[DH : DH + 1, :, :])
                        rec = pb.tile([1, 2, SBLK], F32, tag="rec", bufs=2)
                        nc.vector.reciprocal_approx_fast(rec, den0)
                        bc = pb.tile([DH, 2, SBLK], F32, tag="bc", bufs=2)
                        nc.gpsimd.partition_broadcast(bc, rec)
                        for h01 in range(2):
                            base = h01 * DH
                            o_slice = o_b[ib][base : base + DH, hp, :]
                            nc.vector.tensor_mul(
                                o_slice, o_un[0:DH, h01, :], bc[:, h01, :]
                            )
                        del ps_o_of[pair]
                        c = state["chunk"]
                        if ib < NB - 1:
                            if hp == NT - 1:
                                for u in range(SS * (D // SBLK)):
                                    st = ib * SS + u // (D // SBLK)
                                    nb = u % (D // SBLK)
                                    drip.append((
                                        c + 4,
                                        make_outproj(ib, st, nb, 0, NT,
                                                     out_d, None),
                                    ))
                        else:
                            # last block: split by head-pair to shrink the
                            # tail — t=0 as soon as hp0's normalize lands,
                            # t=1 to out1 at the very end.
                            for u in range(SS * (D // SBLK)):
                                st = ib * SS + u // (D // SBLK)
                                nb = u % (D // SBLK)
                                if hp == 0:
                                    drip.append((
                                        c + 4,
                                        make_outproj(ib, st, nb, 0, 1,
                                                     out_d, None),
                                    ))
                                else:
                                    drip.append((
                                        c + 1,
                                        make_outproj(ib, st, nb, 1, NT,
                                                     out1_d,
                                                     (ib * SS) * P),
                                    ))

                    def make_outproj(ib, st, nb, t0, t1, dst_d, row_off):
                        def emit():
                            pso = psb.tile([P, 2, SBLK], F32, tag="ps_s",
                                           bufs=3, name="pso")[:, 0, :]
                            for t in range(t0, t1):
                                ss_off = (st - ib * SS) * P
                                nc.tensor.matmul(
                                    pso,
                                    lhsT=o_b[ib][:, t, ss_off : ss_off + P],
                                    rhs=wo_sb[:, t,
                                              nb * SBLK : (nb + 1) * SBLK],
                                    start=(t == t0),
                                    stop=(t == t1 - 1),
                                )
                            ob = pb.tile([P, SBLK], BF16, tag="ob", bufs=3)
                            nc.vector.tensor_copy(ob, pso)
                            r0 = st * P - (row_off or 0)
                            nc.sync.dma_start(
                                dst_d[r0 : r0 + P,
                                      nb * SBLK : (nb + 1) * SBLK],
                                ob,
                            )
                        return emit

                    def make_qproj(blk, xt):
                        def emit():
                            psp = psb.tile([P, 2, SBLK], F32, tag="ps_s",
                                           bufs=3, name="qp")
                            for kd in range(KD):
                                for nt in range(NT):
                                    nc.tensor.matmul(
                                        psp[:, nt, :],
                                        lhsT=wq_sb[:, kd,
                                                   nt * P : (nt + 1) * P],
                                        rhs=xt[:, kd, :],
                                        start=(kd == 0),
                                        stop=(kd == KD - 1),
                                    )
                            for nt in range(NT):
                                nc.vector.tensor_scalar_add(
                                    qT_b[blk][:, nt, :], psp[:, nt, :],
                                    bq_sb[:, nt : nt + 1],
                                )
                        return emit

                    for i, (blk, xt) in enumerate(xq_drip):
                        drip.append((4 + 16 * i, make_qproj(blk, xt)))

                    pipe = []
                    for ib in range(NB):
                        for hp in range(NT):
                            pair = (ib, hp)
                            for jt in range(JT):
                                ps_s = psb.tile([P, 2, SBLK], F32,
                                                tag="ps_s", bufs=3,
                                                name="ps_s")
                                for h01 in range(2):
                                    base = h01 * DH
                                    nc.tensor.matmul(
                                        ps_s[:, h01, :],
                                        lhsT=kT[base : base + DH, hp,
                                                jt * P : (jt + 1) * P],
                                        rhs=qT_b[ib][base : base + DH,
                                                     hp, :],
                                        start=True,
                                        stop=True,
                                        tile_position=(base, 0),
                                    )
                                state["chunk"] += 1
                                if len(pipe) == 2:
                                    emit_exp_pv(*pipe.pop(0))
                                if drip and drip[0][0] <= state["chunk"]:
                                    drip.pop(0)[1]()
                                pipe.append((pair, jt, ps_s))
                    while pipe:
                        emit_exp_pv(*pipe.pop(0))
                    while drip:
                        drip.pop(0)[1]()

    nc.finalize()
    return nc


_NC_CACHE = {}


def _get_program(S, D, NH):
    key = (S, D, NH)
    if key not in _NC_CACHE:
        _NC_CACHE[key] = build_core_program(S, D, NH)
    return _NC_CACHE[key]


def make_in_maps(q, k, v, Wq, bq, Wk, bk, Wv, bv, Wo):
    """Host-side sharding: transpose+cast x to bf16 once per batch element,
    slice weights per head-group. Returns the per-core input maps."""
    import ml_dtypes

    bf16 = ml_dtypes.bfloat16
    B, S, D = q.shape
    GROUPS = N_CORES // B
    NSL = D // GROUPS

    xqt = [np.asarray(q[b], np.float32).T.astype(bf16) for b in range(B)]
    xkt = [np.asarray(k[b], np.float32).T.astype(bf16) for b in range(B)]
    xvt = [np.asarray(v[b], np.float32).T.astype(bf16) for b in range(B)]
    Wq, Wk, Wv, Wo = (np.asarray(x, np.float32) for x in (Wq, Wk, Wv, Wo))
    bq, bk = (np.asarray(x, np.float32) for x in (bq, bk))

    in_maps = []
    for c in range(N_CORES):
        b, hg = c // GROUPS, c % GROUPS
        sl = slice(hg * NSL, (hg + 1) * NSL)
        in_maps.append(
            {
                "xqt": xqt[b],
                "xkt": xkt[b],
                "xvt": xvt[b],
                "wq": Wq[:, sl].astype(bf16),
                "wk": Wk[:, sl].astype(bf16),
                "wv": Wv[:, sl].astype(bf16),
                "wo": np.ascontiguousarray(Wo[sl, :]).astype(bf16),
                "bq": np.ascontiguousarray(bq[sl]),
                "bk": np.ascontiguousarray(bk[sl]),
            }
        )
    return in_maps


def kernel(q, k, v, Wq, bq, Wk, bk, Wv, bv, Wo, bo):
    B, S, D = q.shape
    GROUPS = N_CORES // B
    NSL = D // GROUPS
    SBLK = 512 if S % 512 == 0 else S

    nc = _get_program(S, D, NSL // DH)
    in_maps = make_in_maps(q, k, v, Wq, bq, Wk, bk, Wv, bv, Wo)
    res = run_bass_kernel_spmd(nc, in_maps, list(range(N_CORES)))

    out = np.zeros((B, S, D), np.float32)
    for c in range(N_CORES):
        b = c // GROUPS
        out[b] += np.asarray(res.results[c]["out"], np.float32)
        out[b, S - SBLK :] += np.asarray(res.results[c]["out1"], np.float32)
    # bv commutes through the softmax (rows sum to 1): P @ (V + bv) @ Wo =
    # P @ V @ Wo + bv @ Wo. Apply bv@Wo + bo host-side.
    bias = np.asarray(bv, np.float32) @ np.asarray(Wo, np.float32)
    bias += np.asarray(bo, np.float32)
    out += bias[None, None, :]
    return out


# revision 8
# speedup vs baseline: 1.8136x; 1.0324x over previous
"""Multi-head attention (B=2, S=2048, D=1024, H=16) on 8 Trainium2 cores.

Sharding: core c = (batch b, head-group hg) with b = c // 4, hg = c % 4.
Each core computes 4 heads of one batch element end-to-end and emits a
partial output projection; the host sums the 4 partials per batch and adds
bo + bv @ Wo (the V-bias commutes through softmax-normalized attention).

Performance-critical structure (v6):
  - ALL device inputs are pre-packed on the host into partition-major
    layouts (x as [NB*128, KD*SBLK] bf16 blocks, weights as [128, kd*n])
    so every DMA is one large contiguous descriptor per partition — the
    descriptor-fragmented rearranges cost a 16 us DMA head and ~30% of
    aggregate bandwidth in earlier revisions.
  - All matmul operands bf16; PSUM fp32. Projection accumulation chains
    interleave across PSUM banks (4-way for the 256-free V chains) so the
    accumulate turnaround never serializes the PE.
  - Phase B is paced by ScalarE exp (16.8M exps/core, ~1.14 us per
    [128,1024] tile = the hard floor). Score chunks are per-j-tile with both
    heads packed in one 2-bank PSUM tile (3 rotate; PV accumulators pin 2),
    exp->PV runs 2 chunks deep through a global chunk stream that crosses
    head-pair/block boundaries, and softmax normalize runs entirely on
    DVE+GPSIMD (fast reciprocal + partition broadcast).
  - q-projections for blocks 1..3 (split per head-pair) and finished
    blocks' out-projections drip into the chunk stream as PE filler.
  - Tail: the last block's out-projection splits by head-pair (t=0 early
    into `out`, t=1 into `out1`, summed host-side) and its normalize runs
    per-head directly from PSUM to shorten the critical chain.
"""

import numpy as np

import concourse.bacc as bacc
import concourse.mybir as mybir
import concourse.tile as tile
from concourse.bass_utils import run_bass_kernel_spmd

F32 = mybir.dt.float32
BF16 = mybir.dt.bfloat16

S_FULL, D_FULL, NH_PER_CORE, DH = 2048, 1024, 4, 64
N_CORES = 8
B_FULL, H_FULL = 2, 16


def build_core_program(S=S_FULL, D=D_FULL, NH=NH_PER_CORE):
    """One core's program. Packed inputs (bf16 unless noted):
      xqt/xkt/xvt [NB*P, KD*SBLK]  x^T blocks, partition-major
      wq/wk/wv    [P, KD*NSL]      projection weights, partition-major
      wo          [P, NT*D]        out-proj weights, partition-major
      bq/bk       [P, NT] f32
    Outputs: out [S,D] (partial; last block rows hold only the t=0 half)
    and out1 [SBLK,D] (last block t=1 half)."""
    NSL = NH * DH
    P = 128
    KD = D // P
    NT = NSL // P
    SBLK = 512 if S % 512 == 0 else S
    NB = S // SBLK
    JT = S // P
    SS = SBLK // P

    nc = bacc.Bacc("TRN2", target_bir_lowering=False, debug=False)

    xq_d = nc.dram_tensor("xqt", [NB * P, KD * SBLK], BF16,
                          kind="ExternalInput")
    xk_d = nc.dram_tensor("xkt", [NB * P, KD * SBLK], BF16,
                          kind="ExternalInput")
    xv_d = nc.dram_tensor("xvt", [NB * P, KD * SBLK], BF16,
                          kind="ExternalInput")
    wq_d = nc.dram_tensor("wq", [P, KD * NSL], BF16, kind="ExternalInput")
    wk_d = nc.dram_tensor("wk", [P, KD * NSL], BF16, kind="ExternalInput")
    wv_d = nc.dram_tensor("wv", [P, KD * NSL], BF16, kind="ExternalInput")
    wo_d = nc.dram_tensor("wo", [P, NT * D], BF16, kind="ExternalInput")
    bq_d = nc.dram_tensor("bq", [P, NT], F32, kind="ExternalInput")
    bk_d = nc.dram_tensor("bk", [P, NT], F32, kind="ExternalInput")
    out_d = nc.dram_tensor("out", [S, D], BF16, kind="ExternalOutput")
    out1_d = nc.dram_tensor("out1", [SBLK, D], BF16, kind="ExternalOutput")

    def xblk(x_d, blk):
        return x_d[blk * P : (blk + 1) * P, :].rearrange(
            "p (kd s) -> p kd s", kd=KD
        )

    with tile.TileContext(nc) as tc:
        with tc.tile_pool(name="persist", bufs=1) as pp:
            wv_sb = pp.tile([P, KD, NSL], BF16)
            nc.sync.dma_start(
                wv_sb, wv_d.rearrange("p (kd n) -> p kd n", kd=KD)
            )
            wk_sb = pp.tile([P, KD, NSL], BF16)
            wq_sb = pp.tile([P, KD, NSL], BF16)
            wo_sb = pp.tile([P, NT, D], BF16)
            bq_sb = pp.tile([P, NT], F32)
            bk_sb = pp.tile([P, NT], F32)

            qT_b = [
                pp.tile([P, NT, SBLK], BF16, name=f"qT{b}") for b in range(NB)
            ]
            kT = pp.tile([P, NT, S], BF16)
            v_sb = pp.tile([P, JT, NH, DH + 1], BF16)
            nc.vector.memset(v_sb[:, :, :, DH : DH + 1], 1.0)
            o_b = [
                pp.tile([P, NT, SBLK], BF16, name=f"o{b}") for b in range(NB)
            ]

            with tc.tile_pool(name="stage", bufs=3) as pa:
                # ---- Phase A: v + k projections for all blocks, q block 0.
                with tc.tile_pool(name="psa", bufs=2, space="PSUM") as psa:
                    for kind, x_d in (("v", xv_d), ("k", xk_d), ("q0", xq_d)):
                        blks = range(NB) if kind != "q0" else range(1)
                        for blk in blks:
                            xt = pa.tile([P, KD, SBLK], BF16, tag="xt")
                            nc.sync.dma_start(xt, xblk(x_d, blk))
                            if kind == "v" and blk == 0:
                                nc.sync.dma_start(
                                    wk_sb,
                                    wk_d.rearrange("p (kd n) -> p kd n",
                                                   kd=KD),
                                )
                                nc.sync.dma_start(
                                    wq_sb,
                                    wq_d.rearrange("p (kd n) -> p kd n",
                                                   kd=KD),
                                )
                                nc.sync.dma_start(
                                    wo_sb,
                                    wo_d.rearrange("p (t n) -> p t n", t=NT),
                                )
                                nc.sync.dma_start(bq_sb, bq_d[:, :])
                                nc.sync.dma_start(bk_sb, bk_d[:, :])
                            if kind == "v":
                                # 4-way bank interleave: at full clock a
                                # 256-cycle matmul is shorter than the
                                # accumulate turnaround, so 2-way is not
                                # enough to keep the chains pipelined.
                                psv = [
                                    psa.tile([P, NSL], F32, tag="psv",
                                             bufs=4, name=f"psv{i}")
                                    for i in range(SS)
                                ]
                                for kd in range(KD):
                                    for ss in range(SS):
                                        nc.tensor.matmul(
                                            psv[ss],
                                            lhsT=xt[:, kd,
                                                    ss * P : (ss + 1) * P],
                                            rhs=wv_sb[:, kd, :],
                                            start=(kd == 0),
                                            stop=(kd == KD - 1),
                                        )
                                for ss in range(SS):
                                    st = blk * SS + ss
                                    nc.vector.tensor_copy(
                                        v_sb[:, st, :, 0:DH],
                                        psv[ss].rearrange(
                                            "p (h d) -> p h d", d=DH
                                        ),
                                    )
                            else:
                                w_sb = wk_sb if kind == "k" else wq_sb
                                b_sb = bk_sb if kind == "k" else bq_sb
                                psp = [
                                    psa.tile([P, SBLK], F32, tag="psp",
                                             name=f"psp{i}")
                                    for i in range(NT)
                                ]
                                for kd in range(KD):
                                    for nt in range(NT):
                                        nc.tensor.matmul(
                                            psp[nt],
                                            lhsT=w_sb[:, kd,
                                                      nt * P : (nt + 1) * P],
                                            rhs=xt[:, kd, :],
                                            start=(kd == 0),
                                            stop=(kd == KD - 1),
                                        )
                                for nt in range(NT):
                                    dst = (
                                        qT_b[blk][:, nt, :]
                                        if kind == "q0"
                                        else kT[:, nt,
                                                blk * SBLK : (blk + 1) * SBLK]
                                    )
                                    nc.vector.tensor_scalar_add(
                                        dst, psp[nt], b_sb[:, nt : nt + 1]
                                    )

                # xq blocks 1..3 staged for the phase-B q-projection drip
                xq_drip = []
                for blk in range(1, NB):
                    xt = pa.tile([P, KD, SBLK], BF16, tag="xqd")
                    nc.sync.dma_start(xt, xblk(xq_d, blk))
                    xq_drip.append((blk, xt))

                # ---- Phase B: global chunk stream over (ib, hp, jt) ----
                with tc.tile_pool(name="phb", bufs=2) as pb, \
                     tc.tile_pool(name="psb", bufs=1, space="PSUM") as psb:

                    state = {"chunk": 0}
                    drip = []      # (ready_chunk, emit_fn)
                    ps_o_of = {}   # pair -> [ps_o0, ps_o1]

                    def get_ps_o(pair):
                        if pair not in ps_o_of:
                            ps_o_of[pair] = [
                                psb.tile([P, SBLK], F32, tag=f"ps_o{h01}",
                                         bufs=1, name=f"ps_o{h01}")
                                for h01 in range(2)
                            ]
                        return ps_o_of[pair]

                    def emit_exp_pv(pair, jt, ps_s):
                        ib, hp = pair
                        ps_o = get_ps_o(pair)
                        p_sb = pb.tile([P, 2, SBLK], BF16, tag="p_sb",
                                       bufs=4, name="p_sb")
                        nc.scalar.activation(
                            p_sb, ps_s,
                            mybir.ActivationFunctionType.Exp,
                            scale=float(1.0 / np.sqrt(DH)),
                        )
                        for h01 in range(2):
                            h = hp * 2 + h01
                            nc.tensor.matmul(
                                ps_o[h01][0 : DH + 1, :],
                                lhsT=v_sb[:, jt, h, :],
                                rhs=p_sb[:, h01, :],
                                start=(jt == 0),
                                stop=(jt == JT - 1),
                            )
                        if jt == JT - 1:
                            emit_normalize(pair)

                    def emit_normalize(pair):
                        ib, hp = pair
                        ps_o = ps_o_of[pair]
                        last = ib == NB - 1 and hp == NT - 1
                        if not last:
                            o_un = pb.tile([DH + 1, 2, SBLK], F32,
                                           tag="o_un", bufs=2)
                            for h01 in range(2):
                                nc.vector.tensor_copy(
                                    o_un[:, h01, :], ps_o[h01][0 : DH + 1, :]
                                )
                            den0 = pb.tile([1, 2, SBLK], F32, tag="den0",
                                           bufs=2)
                            nc.vector.tensor_copy(
                                den0, o_un[DH : DH + 1, :, :]
                            )
                            rec = pb.tile([1, 2, SBLK], F32, tag="rec",
                                          bufs=2)
                            nc.vector.reciprocal_approx_fast(rec, den0)
                            bc = pb.tile([DH, 2, SBLK], F32, tag="bc",
                                         bufs=2)
                            nc.gpsimd.partition_broadcast(bc, rec)
                            for h01 in range(2):
                                base = h01 * DH
                                o_slice = o_b[ib][base : base + DH, hp, :]
                                nc.vector.tensor_mul(
                                    o_slice, o_un[0:DH, h01, :],
                                    bc[:, h01, :]
                                )
                        else:
                            # tail pair: per-head chains straight from PSUM
                            # (shortest serial latency; no WAR pressure after
                            # this point).
                            for h01 in range(2):
                                base = h01 * DH
                                dn = pb.tile([1, SBLK], F32,
                                             tag=f"dn{h01}", bufs=1,
                                             name=f"dn{h01}")
                                nc.vector.tensor_copy(
                                    dn, ps_o[h01][DH : DH + 1, :]
                                )
                                rc = pb.tile([1, SBLK], F32,
                                             tag=f"rc{h01}", bufs=1,
                                             name=f"rc{h01}")
                                nc.vector.reciprocal_approx_fast(rc, dn)
                                bch = pb.tile([DH, SBLK], F32,
                                              tag=f"bch{h01}", bufs=1,
                                              name=f"bch{h01}")
                                nc.gpsimd.partition_broadcast(bch, rc)
                                o_slice = o_b[ib][base : base + DH, hp, :]
                                nc.vector.tensor_mul(
                                    o_slice, ps_o[h01][0:DH, :], bch
                                )
                        del ps_o_of[pair]
                        c = state["chunk"]
                        if ib < NB - 1:
                            if hp == NT - 1:
                                for u in range(SS * (D // SBLK)):
                                    st = ib * SS + u // (D // SBLK)
                                    nb = u % (D // SBLK)
                                    drip.append((
                                        c + 4,
                                        make_outproj(ib, st, nb, 0, NT,
                                                     out_d, None),
                                    ))
                        else:
                            for u in range(SS * (D // SBLK)):
                                st = ib * SS + u // (D // SBLK)
                                nb = u % (D // SBLK)
                                if hp == 0:
                                    drip.append((
                                        c + 2,
                                        make_outproj(ib, st, nb, 0, 1,
                                                     out_d, None),
                                    ))
                                else:
                                    drip.append((
                                        c,
                                        make_outproj(ib, st, nb, 1, NT,
                                                     out1_d,
                                                     (ib * SS) * P),
                                    ))

                    def make_outproj(ib, st, nb, t0, t1, dst_d, row_off):
                        def emit():
                            pso = psb.tile([P, 2, SBLK], F32, tag="ps_s",
                                           bufs=3, name="pso")[:, 0, :]
                            for t in range(t0, t1):
                                ss_off = (st - ib * SS) * P
                                nc.tensor.matmul(
                                    pso,
                                    lhsT=o_b[ib][:, t, ss_off : ss_off + P],
                                    rhs=wo_sb[:, t,
                                              nb * SBLK : (nb + 1) * SBLK],
                                    start=(t == t0),
                                    stop=(t == t1 - 1),
                                )
                            ob = pb.tile([P, SBLK], BF16, tag="ob", bufs=3)
                            nc.vector.tensor_copy(ob, pso)
                            r0 = st * P - (row_off or 0)
                            nc.sync.dma_start(
                                dst_d[r0 : r0 + P,
                                      nb * SBLK : (nb + 1) * SBLK],
                                ob,
                            )
                        return emit

                    def make_qproj(blk, xt, nt):
                        def emit():
                            psp = psb.tile([P, 2, SBLK], F32, tag="ps_s",
                                           bufs=3, name="qp")
                            for kd in range(KD):
                                nc.tensor.matmul(
                                    psp[:, nt, :],
                                    lhsT=wq_sb[:, kd,
                                               nt * P : (nt + 1) * P],
                                    rhs=xt[:, kd, :],
                                    start=(kd == 0),
                                    stop=(kd == KD - 1),
                                )
                            nc.vector.tensor_scalar_add(
                                qT_b[blk][:, nt, :], psp[:, nt, :],
                                bq_sb[:, nt : nt + 1],
                            )
                        return emit

                    for i, (blk, xt) in enumerate(xq_drip):
                        for nt in range(NT):
                            drip.append((
                                4 + 16 * i + 6 * nt,
                                make_qproj(blk, xt, nt),
                            ))

                    pipe = []
                    for ib in range(NB):
                        for hp in range(NT):
                            pair = (ib, hp)
                            for jt in range(JT):
                                ps_s = psb.tile([P, 2, SBLK], F32,
                                                tag="ps_s", bufs=3,
                                                name="ps_s")
                                for h01 in range(2):
                                    base = h01 * DH
                                    nc.tensor.matmul(
                                        ps_s[:, h01, :],
                                        lhsT=kT[base : base + DH, hp,
                                                jt * P : (jt + 1) * P],
                                        rhs=qT_b[ib][base : base + DH,
                                                     hp, :],
                                        start=True,
                                        stop=True,
                                        tile_position=(base, 0),
                                    )
                                state["chunk"] += 1
                                if len(pipe) == 2:
                                    emit_exp_pv(*pipe.pop(0))
                                if drip and drip[0][0] <= state["chunk"]:
                                    drip.pop(0)[1]()
                                pipe.append((pair, jt, ps_s))
                    while pipe:
                        emit_exp_pv(*pipe.pop(0))
                    while drip:
                        drip.pop(0)[1]()

    nc.finalize()
    return nc


_NC_CACHE = {}


def _get_program(S, D, NH):
    key = (S, D, NH)
    if key not in _NC_CACHE:
        _NC_CACHE[key] = build_core_program(S, D, NH)
    return _NC_CACHE[key]


def _pack_x(x, bf16, P, KD, SBLK, NB):
    """[S, D] fp32 -> [NB*P, KD*SBLK] bf16, partition-major per block:
    out[blk*P + p, kd*SBLK + s] = x[blk*SBLK + s, kd*P + p]."""
    y = x.astype(bf16)
    z = y.reshape(NB, SBLK, KD, P).transpose(0, 3, 2, 1)
    return np.ascontiguousarray(z).reshape(NB * P, KD * SBLK)


def make_in_maps(q, k, v, Wq, bq, Wk, bk, Wv, bv, Wo):
    """Host-side sharding + packing into the device's partition-major
    layouts. Returns the per-core input maps."""
    import ml_dtypes

    bf16 = ml_dtypes.bfloat16
    B, S, D = q.shape
    GROUPS = N_CORES // B
    NSL = D // GROUPS
    P = 128
    KD = D // P
    NT = NSL // P
    SBLK = 512 if S % 512 == 0 else S
    NB = S // SBLK

    q, k, v = (np.asarray(x, np.float32) for x in (q, k, v))
    xqt = [_pack_x(q[b], bf16, P, KD, SBLK, NB) for b in range(B)]
    xkt = [_pack_x(k[b], bf16, P, KD, SBLK, NB) for b in range(B)]
    xvt = [_pack_x(v[b], bf16, P, KD, SBLK, NB) for b in range(B)]
    Wq, Wk, Wv, Wo = (np.asarray(x, np.float32) for x in (Wq, Wk, Wv, Wo))
    bq, bk = (np.asarray(x, np.float32) for x in (bq, bk))

    def pack_w(w):  # [D, NSL] -> [P, KD*NSL]
        return np.ascontiguousarray(
            w.astype(bf16).reshape(KD, P, NSL).transpose(1, 0, 2)
        ).reshape(P, KD * NSL)

    def pack_wo(w):  # [NSL, D] -> [P, NT*D]
        return np.ascontiguousarray(
            w.astype(bf16).reshape(NT, P, D).transpose(1, 0, 2)
        ).reshape(P, NT * D)

    def pack_b(b):  # [NSL] -> [P, NT]
        return np.ascontiguousarray(b.reshape(NT, P).T)

    in_maps = []
    for c in range(N_CORES):
        b, hg = c // GROUPS, c % GROUPS
        sl = slice(hg * NSL, (hg + 1) * NSL)
        in_maps.append(
            {
                "xqt": xqt[b],
                "xkt": xkt[b],
                "xvt": xvt[b],
                "wq": pack_w(Wq[:, sl]),
                "wk": pack_w(Wk[:, sl]),
                "wv": pack_w(Wv[:, sl]),
                "wo": pack_wo(np.ascontiguousarray(Wo[sl, :])),
                "bq": pack_b(bq[sl]),
                "bk": pack_b(bk[sl]),
            }
        )
    return in_maps


def kernel(q, k, v, Wq, bq, Wk, bk, Wv, bv, Wo, bo):
    B, S, D = q.shape
    GROUPS = N_CORES // B
    NSL = D // GROUPS
    SBLK = 512 if S % 512 == 0 else S

    nc = _get_program(S, D, NSL // DH)
    in_maps = make_in_maps(q, k, v, Wq, bq, Wk, bk, Wv, bv, Wo)
    res = run_bass_kernel_spmd(nc, in_maps, list(range(N_CORES)))

    out = np.zeros((B, S, D), np.float32)
    for c in range(N_CORES):
        b = c // GROUPS
        out[b] += np.asarray(res.results[c]["out"], np.float32)
        out[b, S - SBLK :] += np.asarray(res.results[c]["out1"], np.float32)
    # bv commutes through the softmax (rows sum to 1): P @ (V + bv) @ Wo =
    # P @ V @ Wo + bv @ Wo. Apply bv@Wo + bo host-side.
    bias = np.asarray(bv, np.float32) @ np.asarray(Wo, np.float32)
    bias += np.asarray(bo, np.float32)
    out += bias[None, None, :]
    return out


# revision 10
# speedup vs baseline: 1.8572x; 1.0240x over previous
"""Multi-head attention (B=2, S=2048, D=1024, H=16) on 8 Trainium2 cores.

Sharding: core c = (batch b, head-group hg) with b = c // 4, hg = c % 4.
Each core computes 4 heads of one batch element end-to-end and emits a
partial output projection; the host sums the 4 partials per batch and adds
bo + bv @ Wo (the V-bias commutes through softmax-normalized attention).

Performance-critical structure (v6):
  - ALL device inputs are pre-packed on the host into partition-major
    layouts (x as [NB*128, KD*SBLK] bf16 blocks, weights as [128, kd*n])
    so every DMA is one large contiguous descriptor per partition — the
    descriptor-fragmented rearranges cost a 16 us DMA head and ~30% of
    aggregate bandwidth in earlier revisions.
  - All matmul operands bf16; PSUM fp32. Projection accumulation chains
    interleave across PSUM banks (4-way for the 256-free V chains) so the
    accumulate turnaround never serializes the PE.
  - Phase B is paced by ScalarE exp (16.8M exps/core, ~1.14 us per
    [128,1024] tile = the hard floor). Score chunks are per-j-tile with both
    heads packed in one 2-bank PSUM tile (3 rotate; PV accumulators pin 2),
    exp->PV runs 2 chunks deep through a global chunk stream that crosses
    head-pair/block boundaries, and softmax normalize runs entirely on
    DVE+GPSIMD (fast reciprocal + partition broadcast).
  - q-projections for blocks 1..3 (split per head-pair) and finished
    blocks' out-projections drip into the chunk stream as PE filler.
  - Tail: the last block's out-projection splits by head-pair (t=0 early
    into `out`, t=1 into `out1`, summed host-side) and its normalize runs
    per-head directly from PSUM to shorten the critical chain.
"""

import numpy as np

import concourse.bacc as bacc
import concourse.mybir as mybir
import concourse.tile as tile
from concourse.bass_utils import run_bass_kernel_spmd

F32 = mybir.dt.float32
BF16 = mybir.dt.bfloat16

S_FULL, D_FULL, NH_PER_CORE, DH = 2048, 1024, 4, 64
N_CORES = 8
B_FULL, H_FULL = 2, 16


def build_core_program(S=S_FULL, D=D_FULL, NH=NH_PER_CORE):
    """One core's program. Packed inputs (bf16 unless noted):
      xqt/xkt/xvt [NB*P, KD*SBLK]  x^T blocks, partition-major
      wq/wk/wv    [P, KD*NSL]      projection weights, partition-major
      wo          [P, NT*D]        out-proj weights, partition-major
      bq/bk       [P, NT] f32
    Outputs: out [S,D] (partial; last block rows hold only the t=0 half)
    and out1 [SBLK,D] (last block t=1 half)."""
    NSL = NH * DH
    P = 128
    KD = D // P
    NT = NSL // P
    SBLK = 512 if S % 512 == 0 else S
    NB = S // SBLK
    JT = S // P
    SS = SBLK // P

    nc = bacc.Bacc("TRN2", target_bir_lowering=False, debug=False)

    xq_d = nc.dram_tensor("xqt", [NB * P, KD * SBLK], BF16,
                          kind="ExternalInput")
    xk_d = nc.dram_tensor("xkt", [NB * P, KD * SBLK], BF16,
                          kind="ExternalInput")
    xv_d = nc.dram_tensor("xvt", [NB * P, KD * SBLK], BF16,
                          kind="ExternalInput")
    wq_d = nc.dram_tensor("wq", [P, KD * NSL], BF16, kind="ExternalInput")
    wk_d = nc.dram_tensor("wk", [P, KD * NSL], BF16, kind="ExternalInput")
    wv_d = nc.dram_tensor("wv", [P, KD * NSL], BF16, kind="ExternalInput")
    wo_d = nc.dram_tensor("wo", [P, NT * D], BF16, kind="ExternalInput")
    bq_d = nc.dram_tensor("bq", [P, NT], F32, kind="ExternalInput")
    bk_d = nc.dram_tensor("bk", [P, NT], F32, kind="ExternalInput")
    out_d = nc.dram_tensor("out", [S, D], BF16, kind="ExternalOutput")
    out1_d = nc.dram_tensor("out1", [SBLK, D], BF16, kind="ExternalOutput")

    def xblk(x_d, blk):
        return x_d[blk * P : (blk + 1) * P, :].rearrange(
            "p (kd s) -> p kd s", kd=KD
        )

    with tile.TileContext(nc) as tc:
        with tc.tile_pool(name="persist", bufs=1) as pp:
            wv_sb = pp.tile([P, KD, NSL], BF16)
            nc.sync.dma_start(
                wv_sb, wv_d.rearrange("p (kd n) -> p kd n", kd=KD)
            )
            wk_sb = pp.tile([P, KD, NSL], BF16)
            wq_sb = pp.tile([P, KD, NSL], BF16)
            wo_sb = pp.tile([P, NT, D], BF16)
            bq_sb = pp.tile([P, NT], F32)
            bk_sb = pp.tile([P, NT], F32)

            qT_b = [
                pp.tile([P, NT, SBLK], BF16, name=f"qT{b}") for b in range(NB)
            ]
            kT = pp.tile([P, NT, S], BF16)
            v_sb = pp.tile([P, JT, NH, DH + 1], BF16)
            nc.vector.memset(v_sb[:, :, :, DH : DH + 1], 1.0)
            o_b = [
                pp.tile([P, NT, SBLK], BF16, name=f"o{b}") for b in range(NB)
            ]

            with tc.tile_pool(name="stage", bufs=3) as pa:
                # ---- Phase A: v + k projections for all blocks, q block 0.
                with tc.tile_pool(name="psa", bufs=2, space="PSUM") as psa:
                    for kind, x_d in (("v", xv_d), ("k", xk_d), ("q0", xq_d)):
                        blks = range(NB) if kind != "q0" else range(1)
                        for blk in blks:
                            xt = pa.tile([P, KD, SBLK], BF16, tag="xt")
                            nc.sync.dma_start(xt, xblk(x_d, blk))
                            if kind == "v" and blk == 0:
                                nc.sync.dma_start(
                                    wk_sb,
                                    wk_d.rearrange("p (kd n) -> p kd n",
                                                   kd=KD),
                                )
                                nc.sync.dma_start(
                                    wq_sb,
                                    wq_d.rearrange("p (kd n) -> p kd n",
                                                   kd=KD),
                                )
                                nc.sync.dma_start(
                                    wo_sb,
                                    wo_d.rearrange("p (t n) -> p t n", t=NT),
                                )
                                nc.sync.dma_start(bq_sb, bq_d[:, :])
                                nc.sync.dma_start(bk_sb, bk_d[:, :])
                            if kind == "v":
                                # 4-way bank interleave: at full clock a
                                # 256-cycle matmul is shorter than the
                                # accumulate turnaround, so 2-way is not
                                # enough to keep the chains pipelined.
                                psv = [
                                    psa.tile([P, NSL], F32, tag="psv",
                                             bufs=4, name=f"psv{i}")
                                    for i in range(SS)
                                ]
                                for kd in range(KD):
                                    for ss in range(SS):
                                        nc.tensor.matmul(
                                            psv[ss],
                                            lhsT=xt[:, kd,
                                                    ss * P : (ss + 1) * P],
                                            rhs=wv_sb[:, kd, :],
                                            start=(kd == 0),
                                            stop=(kd == KD - 1),
                                        )
                                for ss in range(SS):
                                    st = blk * SS + ss
                                    nc.vector.tensor_copy(
                                        v_sb[:, st, :, 0:DH],
                                        psv[ss].rearrange(
                                            "p (h d) -> p h d", d=DH
                                        ),
                                    )
                            else:
                                w_sb = wk_sb if kind == "k" else wq_sb
                                b_sb = bk_sb if kind == "k" else bq_sb
                                psp = [
                                    psa.tile([P, SBLK], F32, tag="psp",
                                             name=f"psp{i}")
                                    for i in range(NT)
                                ]
                                for kd in range(KD):
                                    for nt in range(NT):
                                        nc.tensor.matmul(
                                            psp[nt],
                                            lhsT=w_sb[:, kd,
                                                      nt * P : (nt + 1) * P],
                                            rhs=xt[:, kd, :],
                                            start=(kd == 0),
                                            stop=(kd == KD - 1),
                                        )
                                for nt in range(NT):
                                    dst = (
                                        qT_b[blk][:, nt, :]
                                        if kind == "q0"
                                        else kT[:, nt,
                                                blk * SBLK : (blk + 1) * SBLK]
                                    )
                                    nc.vector.tensor_scalar_add(
                                        dst, psp[nt], b_sb[:, nt : nt + 1]
                                    )

                # xq blocks 1..3 staged for the phase-B q-projection drip
                xq_drip = []
                for blk in range(1, NB):
                    xt = pa.tile([P, KD, SBLK], BF16, tag="xqd")
                    nc.sync.dma_start(xt, xblk(xq_d, blk))
                    xq_drip.append((blk, xt))

                # ---- Phase B: global chunk stream over (ib, hp, jt) ----
                with tc.tile_pool(name="phb", bufs=2) as pb, \
                     tc.tile_pool(name="psb", bufs=1, space="PSUM") as psb:

                    state = {"chunk": 0}
                    drip = []      # (ready_chunk, emit_fn)
                    ps_o_of = {}   # pair -> [ps_o0, ps_o1]

                    def get_ps_o(pair):
                        if pair not in ps_o_of:
                            ps_o_of[pair] = [
                                psb.tile([P, SBLK], F32, tag=f"ps_o{h01}",
                                         bufs=1, name=f"ps_o{h01}")
                                for h01 in range(2)
                            ]
                        return ps_o_of[pair]

                    def emit_exp_pv(pair, jt, ps_s):
                        ib, hp = pair
                        ps_o = get_ps_o(pair)
                        p_sb = pb.tile([P, 2, SBLK], BF16, tag="p_sb",
                                       bufs=4, name="p_sb")
                        nc.scalar.activation(
                            p_sb, ps_s,
                            mybir.ActivationFunctionType.Exp,
                            scale=float(1.0 / np.sqrt(DH)),
                        )
                        for h01 in range(2):
                            h = hp * 2 + h01
                            nc.tensor.matmul(
                                ps_o[h01][0 : DH + 1, :],
                                lhsT=v_sb[:, jt, h, :],
                                rhs=p_sb[:, h01, :],
                                start=(jt == 0),
                                stop=(jt == JT - 1),
                            )
                        if jt == JT - 1:
                            emit_normalize(pair)

                    def emit_normalize(pair):
                        ib, hp = pair
                        ps_o = ps_o_of[pair]
                        last = ib == NB - 1 and hp == NT - 1
                        if not last:
                            o_un = pb.tile([DH + 1, 2, SBLK], F32,
                                           tag="o_un", bufs=2)
                            for h01 in range(2):
                                nc.vector.tensor_copy(
                                    o_un[:, h01, :], ps_o[h01][0 : DH + 1, :]
                                )
                            den0 = pb.tile([1, 2, SBLK], F32, tag="den0",
                                           bufs=2)
                            nc.vector.tensor_copy(
                                den0, o_un[DH : DH + 1, :, :]
                            )
                            rec = pb.tile([1, 2, SBLK], F32, tag="rec",
                                          bufs=2)
                            nc.vector.reciprocal_approx_fast(rec, den0)
                            bc = pb.tile([DH, 2, SBLK], F32, tag="bc",
                                         bufs=2)
                            nc.gpsimd.partition_broadcast(bc, rec)
                            for h01 in range(2):
                                base = h01 * DH
                                o_slice = o_b[ib][base : base + DH, hp, :]
                                nc.vector.tensor_mul(
                                    o_slice, o_un[0:DH, h01, :],
                                    bc[:, h01, :]
                                )
                        else:
                            # tail pair: per-head chains straight from PSUM
                            # (shortest serial latency; no WAR pressure after
                            # this point).
                            for h01 in range(2):
                                base = h01 * DH
                                dn = pb.tile([1, SBLK], F32,
                                             tag=f"dn{h01}", bufs=1,
                                             name=f"dn{h01}")
                                nc.vector.tensor_copy(
                                    dn, ps_o[h01][DH : DH + 1, :]
                                )
                                rc = pb.tile([1, SBLK], F32,
                                             tag=f"rc{h01}", bufs=1,
                                             name=f"rc{h01}")
                                nc.vector.reciprocal_approx_fast(rc, dn)
                                bch = pb.tile([DH, SBLK], F32,
                                              tag=f"bch{h01}", bufs=1,
                                              name=f"bch{h01}")
                                nc.gpsimd.partition_broadcast(bch, rc)
                                o_slice = o_b[ib][base : base + DH, hp, :]
                                nc.vector.tensor_mul(
                                    o_slice, ps_o[h01][0:DH, :], bch
                                )
                        del ps_o_of[pair]
                        c = state["chunk"]
                        if ib < NB - 1:
                            if hp == NT - 1:
                                for st in range(ib * SS, (ib + 1) * SS):
                                    drip.append((
                                        c + 4,
                                        make_outproj(ib, st, 0, NT,
                                                     out_d, None, False),
                                    ))
                        else:
                            for st in range(ib * SS, (ib + 1) * SS):
                                if hp == 0:
                                    drip.append((
                                        c + 3,
                                        make_outproj(ib, st, 0, 1,
                                                     out_d, None, False),
                                    ))
                                else:
                                    drip.append((
                                        c,
                                        make_outproj(ib, st, 1, NT,
                                                     out1_d, (ib * SS) * P,
                                                     True),
                                    ))

                    def make_outproj(ib, st, t0, t1, dst_d, row_off, tail):
                        # full-D output chunk: 1024-wide moving dim uses the
                        # whole 2-bank rotation tile in one go (half the
                        # instruction / copy / DMA count of 512-wide chunks).
                        def emit():
                            pso = psb.tile([P, 2, SBLK], F32, tag="ps_s",
                                           bufs=3, name="pso")
                            for t in range(t0, t1):
                                ss_off = (st - ib * SS) * P
                                for a in range(2):
                                    nc.tensor.matmul(
                                        pso[:, a, :],
                                        lhsT=o_b[ib][:, t,
                                                     ss_off : ss_off + P],
                                        rhs=wo_sb[:, t,
                                                  a * SBLK : (a + 1) * SBLK],
                                        start=(t == t0),
                                        stop=(t == t1 - 1),
                                    )
                            ob = pb.tile([P, 2, SBLK], BF16, tag="ob",
                                         bufs=3)
                            if tail:
                                # ScalarE is idle after the last exp — split
                                # the evacuation across both engines.
                                nc.scalar.copy(ob[:, 0, :], pso[:, 0, :])
                                nc.vector.tensor_copy(ob[:, 1, :],
                                                      pso[:, 1, :])
                            else:
                                nc.vector.tensor_copy(ob, pso)
                            r0 = st * P - (row_off or 0)
                            nc.sync.dma_start(
                                dst_d[r0 : r0 + P, :],
                                ob.rearrange("p a s -> p (a s)"),
                            )
                        return emit

                    def make_qproj(blk, xt, nt):
                        def emit():
                            psp = psb.tile([P, 2, SBLK], F32, tag="ps_s",
                                           bufs=3, name="qp")
                            for kd in range(KD):
                                nc.tensor.matmul(
                                    psp[:, nt, :],
                                    lhsT=wq_sb[:, kd,
                                               nt * P : (nt + 1) * P],
                                    rhs=xt[:, kd, :],
                                    start=(kd == 0),
                                    stop=(kd == KD - 1),
                                )
                            nc.vector.tensor_scalar_add(
                                qT_b[blk][:, nt, :], psp[:, nt, :],
                                bq_sb[:, nt : nt + 1],
                            )
                        return emit

                    for i, (blk, xt) in enumerate(xq_drip):
                        for nt in range(NT):
                            drip.append((
                                4 + 16 * i + 6 * nt,
                                make_qproj(blk, xt, nt),
                            ))

                    pipe = []
                    for ib in range(NB):
                        for hp in range(NT):
                            pair = (ib, hp)
                            for jt in range(JT):
                                ps_s = psb.tile([P, 2, SBLK], F32,
                                                tag="ps_s", bufs=3,
                                                name="ps_s")
                                for h01 in range(2):
                                    base = h01 * DH
                                    nc.tensor.matmul(
                                        ps_s[:, h01, :],
                                        lhsT=kT[base : base + DH, hp,
                                                jt * P : (jt + 1) * P],
                                        rhs=qT_b[ib][base : base + DH,
                                                     hp, :],
                                        start=True,
                                        stop=True,
                                        tile_position=(base, 0),
                                    )
                                state["chunk"] += 1
                                if len(pipe) == 2:
                                    emit_exp_pv(*pipe.pop(0))
                                if drip and drip[0][0] <= state["chunk"]:
                                    drip.pop(0)[1]()
                                pipe.append((pair, jt, ps_s))
                    while pipe:
                        emit_exp_pv(*pipe.pop(0))
                    while drip:
                        drip.pop(0)[1]()

    nc.finalize()
    return nc


_NC_CACHE = {}


def _get_program(S, D, NH):
    key = (S, D, NH)
    if key not in _NC_CACHE:
        _NC_CACHE[key] = build_core_program(S, D, NH)
    return _NC_CACHE[key]


def _pack_x(x, bf16, P, KD, SBLK, NB):
    """[S, D] fp32 -> [NB*P, KD*SBLK] bf16, partition-major per block:
    out[blk*P + p, kd*SBLK + s] = x[blk*SBLK + s, kd*P + p]."""
    y = x.astype(bf16)
    z = y.reshape(NB, SBLK, KD, P).transpose(0, 3, 2, 1)
    return np.ascontiguousarray(z).reshape(NB * P, KD * SBLK)


def make_in_maps(q, k, v, Wq, bq, Wk, bk, Wv, bv, Wo):
    """Host-side sharding + packing into the device's partition-major
    layouts. Returns the per-core input maps."""
    import ml_dtypes

    bf16 = ml_dtypes.bfloat16
    B, S, D = q.shape
    GROUPS = N_CORES // B
    NSL = D // GROUPS
    P = 128
    KD = D // P
    NT = NSL // P
    SBLK = 512 if S % 512 == 0 else S
    NB = S // SBLK

    q, k, v = (np.asarray(x, np.float32) for x in (q, k, v))
    xqt = [_pack_x(q[b], bf16, P, KD, SBLK, NB) for b in range(B)]
    xkt = [_pack_x(k[b], bf16, P, KD, SBLK, NB) for b in range(B)]
    xvt = [_pack_x(v[b], bf16, P, KD, SBLK, NB) for b in range(B)]
    Wq, Wk, Wv, Wo = (np.asarray(x, np.float32) for x in (Wq, Wk, Wv, Wo))
    bq, bk = (np.asarray(x, np.float32) for x in (bq, bk))

    def pack_w(w):  # [D, NSL] -> [P, KD*NSL]
        return np.ascontiguousarray(
            w.astype(bf16).reshape(KD, P, NSL).transpose(1, 0, 2)
        ).reshape(P, KD * NSL)

    def pack_wo(w):  # [NSL, D] -> [P, NT*D]
        return np.ascontiguousarray(
            w.astype(bf16).reshape(NT, P, D).transpose(1, 0, 2)
        ).reshape(P, NT * D)

    def pack_b(b):  # [NSL] -> [P, NT]
        return np.ascontiguousarray(b.reshape(NT, P).T)

    in_maps = []
    for c in range(N_CORES):
        b, hg = c // GROUPS, c % GROUPS
        sl = slice(hg * NSL, (hg + 1) * NSL)
        in_maps.append(
            {
                "xqt": xqt[b],
                "xkt": xkt[b],
                "xvt": xvt[b],
                "wq": pack_w(Wq[:, sl]),
                "wk": pack_w(Wk[:, sl]),
                "wv": pack_w(Wv[:, sl]),
                "wo": pack_wo(np.ascontiguousarray(Wo[sl, :])),
                "bq": pack_b(bq[sl]),
                "bk": pack_b(bk[sl]),
            }
        )
    return in_maps


def kernel(q, k, v, Wq, bq, Wk, bk, Wv, bv, Wo, bo):
    B, S, D = q.shape
    GROUPS = N_CORES // B
    NSL = D // GROUPS
    SBLK = 512 if S % 512 == 0 else S

    nc = _get_program(S, D, NSL // DH)
    in_maps = make_in_maps(q, k, v, Wq, bq, Wk, bk, Wv, bv, Wo)
    res = run_bass_kernel_spmd(nc, in_maps, list(range(N_CORES)))

    out = np.zeros((B, S, D), np.float32)
    for c in range(N_CORES):
        b = c // GROUPS
        out[b] += np.asarray(res.results[c]["out"], np.float32)
        out[b, S - SBLK :] += np.asarray(res.results[c]["out1"], np.float32)
    # bv commutes through the softmax (rows sum to 1): P @ (V + bv) @ Wo =
    # P @ V @ Wo + bv @ Wo. Apply bv@Wo + bo host-side.
    bias = np.asarray(bv, np.float32) @ np.asarray(Wo, np.float32)
    bias += np.asarray(bo, np.float32)
    out += bias[None, None, :]
    return out
